# revision 15
# baseline (speedup 1.0000x reference)
"""Multi-head attention (B=8, N=1024, C=768, H=12) on 8 Trainium2 NeuronCores.

Strategy: pure data parallelism over the batch dimension — each of the 8
cores computes full attention for one batch element; weights are
replicated. No collectives needed.

Per-core dataflow (all matmuls expressed as out = lhsT.T @ rhs on the PE):
  1. xT  = transpose(x)                          (PE identity-transpose, 48 blocks)
  2. qkT = w_qkv[:, :1536].T @ xT  (q,k feature-major)   lhsT = w_qkv chunks
     v   = x @ w_qkv[:, 1536:]    (v token-major)        lhsT = xT chunks
  3. per head pair (2 heads share a 128-row qkT chunk → row-tiled K=64 matmuls):
       scoresT[m,n] = k_h @ q_h^T   (lhsT = kT slice, rhs = qT slice)
       expT = exp(scale * scoresT)  (ScalarE, softmax max-subtraction skipped:
                                     |scores*scale| < ~2, exp is safe in fp32)
       U[n, 0:65] += expT[mchunk].T @ [v_h | 1]  (ones column gives the softmax
                                                  denominator in U[:, 64])
       attn_out[n, h*64:(h+1)*64] = U[:, :64] * (1/U[:, 64])
  4. attn_outT = transpose(attn_out); y = attn_outT.T @ w_proj + b
"""

import os
import sys

for _p in ("/opt/trn_rl_repo", "/root/.axon_site/_ro/trn_rl_repo"):
    if os.path.isdir(_p) and _p not in sys.path:
        sys.path.append(_p)

from contextlib import ExitStack

import numpy as np

import concourse.bass as bass
import concourse.tile as tile
from concourse import bacc, mybir
from concourse.bass_utils import run_bass_kernel_spmd
from concourse.masks import make_identity

FP = mybir.dt.float32
BF16 = mybir.dt.bfloat16
F32R = mybir.dt.float32r
N_CORES = 8
T = 1024  # tokens per core (batch element)
C = 768
H = 12
D = 64
SCALE = D ** (-0.5)
TC = T // 128  # 8 token chunks
CCH = C // 128  # 6 channel chunks
NPAIR = H // 2  # 6 head pairs

Exp = mybir.ActivationFunctionType.Exp


def build(n_cores: int = N_CORES, fast: bool = True):
    # fast=True: run the large matmuls (qkv / scores / proj, all N>=256) with
    # operands tagged float32r — the PE's full-rate fp32 mode (1 cycle/row vs
    # 4 for plain fp32). Storage is IEEE fp32 either way; only the matmul
    # interpretation changes.
    MMDT = F32R if fast else FP
    nc = bacc.Bacc(
        "TRN2", target_bir_lowering=False, debug=False, num_devices=n_cores
    )
    wdma = nc.gpsimd.dma_start if fast else nc.sync.dma_start
    x = nc.declare_dram_parameter("x", [T, C], FP, isOutput=False)
    w_qkv = nc.declare_dram_parameter("w_qkv", [C, 3 * C], FP, isOutput=False)
    w_proj = nc.declare_dram_parameter("w_proj", [C, C], FP, isOutput=False)
    b_proj = nc.declare_dram_parameter("b_proj", [C], FP, isOutput=False)
    out = nc.declare_dram_parameter("out", [T, C], FP, isOutput=True)

    xa, wqa, wpa, outa = x.ap(), w_qkv.ap(), w_proj.ap(), out.ap()
    ba = b_proj.ap()
    b_bcast_src = bass.AP(tensor=ba.tensor, offset=ba.offset, ap=[[0, 128]] + ba.ap)

    with tile.TileContext(nc) as tc, ExitStack() as ctx:
        # ---- persistent pools (live for the whole kernel) ----
        consts = ctx.enter_context(tc.tile_pool(name="consts", bufs=1))
        qk_pool = ctx.enter_context(tc.tile_pool(name="qk", bufs=12))
        v_pool = ctx.enter_context(tc.tile_pool(name="v65", bufs=TC))
        ao_pool = ctx.enter_context(tc.tile_pool(name="attn_out", bufs=TC))
        wp_pool = ctx.enter_context(tc.tile_pool(name="wp", bufs=CCH))
        y_pool = ctx.enter_context(tc.tile_pool(name="y", bufs=2))
        r_pool = ctx.enter_context(tc.tile_pool(name="r", bufs=4))

        identity = consts.tile([128, 128], FP)
        make_identity(nc, identity)
        ones_h = consts.tile([128, 2 * H], FP)
        nc.vector.memset(ones_h[:], 1.0)

        v65 = [v_pool.tile([128, H, 128], BF16 if fast else FP, tag="v65", name="v65") for _ in range(TC)]
        attn_out = [ao_pool.tile([128, C], FP, tag="ao", name="ao") for _ in range(TC)]
        qkT = [qk_pool.tile([128, T], BF16 if fast else FP, tag="qk", name="qk") for _ in range(12)]

        # ================= phase 1: xT, qkT, v =================
        with (
            tc.tile_pool(name="xstage", bufs=2) as xs_pool,
            tc.tile_pool(name="xT", bufs=CCH) as xT_pool,
            tc.tile_pool(name="mm1", bufs=4, space="PSUM") as mm1,
            tc.tile_pool(name="tp1", bufs=2, space="PSUM") as tp1,
        ):
            xT = [xT_pool.tile([128, T], MMDT, tag="xT", name="xT") for _ in range(CCH)]
            for t in range(TC):
                xs = xs_pool.tile([128, C], FP, tag="xs")
                nc.sync.dma_start(xs[:], xa[t * 128 : (t + 1) * 128, :])
                for c in range(CCH):
                    ps = tp1.tile([128, 128], FP, tag="tp")
                    nc.tensor.transpose(
                        ps[:], xs[:, c * 128 : (c + 1) * 128], identity[:]
                    )
                    nc.vector.tensor_copy(xT[c][:, t * 128 : (t + 1) * 128], ps[:])

            # family 2: v token-major, interleaved with ones column
            with tc.tile_pool(name="wq2", bufs=CCH) as wq2_pool:
                wq2 = []
                for c in range(CCH):
                    w = wq2_pool.tile([128, C], MMDT, tag="wq2")
                    wdma(
                        w[:], wqa[c * 128 : (c + 1) * 128, 2 * C : 3 * C]
                    )
                    wq2.append(w)
                for t in range(TC):
                    nc.vector.memset(v65[t][:, :, D : D + 2], 1.0)
                    for nh in range(2):
                        ps = mm1.tile([128, 384], FP, tag="mm")
                        for c in range(CCH):
                            nc.tensor.matmul(
                                ps[:],
                                xT[c][:, t * 128 : (t + 1) * 128],
                                wq2[c][:, nh * 384 : (nh + 1) * 384],
                                start=(c == 0),
                                stop=(c == CCH - 1),
                            )
                        nc.vector.tensor_copy(
                            v65[t][:, nh * 6 : (nh + 1) * 6, 0:D],
                            ps.rearrange("p (g d) -> p g d", g=6),
                        )

            # family 1: qT/kT feature-major (qkT rows 0:1536)
            with tc.tile_pool(name="wq1", bufs=CCH) as wq1_pool:
                wq1 = []
                for c in range(CCH):
                    w = wq1_pool.tile([128, 2 * C], MMDT, tag="wq1")
                    wdma(w[:], wqa[c * 128 : (c + 1) * 128, 0 : 2 * C])
                    wq1.append(w)
                for j in (0, 6, 1, 7, 2, 8, 3, 9, 4, 10, 5, 11):
                    for nh in range(2):
                        ps = mm1.tile([128, 512], FP, tag="mm")
                        for c in range(CCH):
                            nc.tensor.matmul(
                                ps[:],
                                wq1[c][:, j * 128 : (j + 1) * 128],
                                xT[c][:, nh * 512 : (nh + 1) * 512],
                                start=(c == 0),
                                stop=(c == CCH - 1),
                            )
                        nc.any.tensor_copy(
                            qkT[j][:, nh * 512 : (nh + 1) * 512], ps[:]
                        )

        # proj weights + bias: only needed in phase 3; issue DMAs after the
        # phase-1 weight loads so they don't starve the first matmuls
        wp = []
        for c in range(CCH):
            w = wp_pool.tile([128, C], MMDT, tag="wp")
            wdma(w[:], wpa[c * 128 : (c + 1) * 128, :])
            wp.append(w)
        b_bcast = consts.tile([128, C], FP)
        nc.sync.dma_start(b_bcast[:], b_bcast_src)

        # ================= phase 2: attention =================
        with (
            tc.tile_pool(name="expT", bufs=2) as exp_pool,
            tc.tile_pool(name="uT", bufs=2) as uT_pool,
            tc.tile_pool(name="sc", bufs=2, space="PSUM") as sc_psum,
            tc.tile_pool(name="u", bufs=2, space="PSUM") as u_psum,
            tc.tile_pool(name="tpu", bufs=2, space="PSUM") as tpu_psum,
        ):
            for p in range(NPAIR):
                eAB = [
                    exp_pool.tile([128, TC, T], BF16 if fast else FP, tag="expT", name="expT") for _ in range(2)
                ]
                for j in range(TC):  # key-token chunks (m)
                    psAB = [
                        sc_psum.tile([128, T], FP, tag="sc", name="sc") for _ in range(2)
                    ]
                    for half in range(2):
                        base = 64 * half
                        for nh in range(2):
                            nc.tensor.matmul(
                                psAB[half][:, nh * 512 : (nh + 1) * 512],
                                qkT[6 + p][
                                    base : base + 64, j * 128 : (j + 1) * 128
                                ],
                                qkT[p][base : base + 64, nh * 512 : (nh + 1) * 512],
                                start=True,
                                stop=True,
                            )
                        nc.scalar.activation(
                            eAB[half][:, j, :], psAB[half][:], Exp, scale=SCALE
                        )
                for half in range(2):
                    h = 2 * p + half
                    e = eAB[half]
                    # U^T[d, n] = sum_m v_aug[m, d] * expT[m, n]; v stationary,
                    # expT moving at N=512 (f32r full rate). Row 64 holds the
                    # softmax denominator via the ones column of v_aug.
                    uT_sb = uT_pool.tile([D + 2, T], FP, tag="uT", name="uT")
                    upsAB = [
                        u_psum.tile([128, 512], FP, tag="u", name="u")
                        for _ in range(2)
                    ]
                    for j in range(TC):  # lhsT (v) reused across both halves
                        for nh in range(2):
                            nc.tensor.matmul(
                                upsAB[nh][:],
                                v65[j][:, h, :],
                                e[:, j, nh * 512 : (nh + 1) * 512],
                                start=(j == 0),
                                stop=(j == TC - 1),
                            )
                    for nh in range(2):
                        nc.vector.tensor_copy(
                            uT_sb[:, nh * 512 : (nh + 1) * 512],
                            upsAB[nh][0 : D + 2, :],
                        )
                    # transpose U^T back to token-major per 128-token chunk,
                    # then normalize with per-partition reciprocal of row 64
                    for i in range(TC):
                        tps = tpu_psum.tile([128, D + 2], FP, tag="tpu", name="tpu")
                        nc.tensor.transpose(
                            tps[:],
                            uT_sb[:, i * 128 : (i + 1) * 128],
                            identity[0 : D + 2, 0 : D + 2],
                        )
                        r = r_pool.tile([128, 1], FP, tag="r")
                        nc.vector.reciprocal(r[:], tps[:, D : D + 1])
                        nc.vector.tensor_scalar_mul(
                            attn_out[i][:, h * D : (h + 1) * D], tps[:, 0:D], r[:]
                        )

        # ================= phase 3: attn_outT, proj =================
        with (
            tc.tile_pool(name="aoT", bufs=CCH) as aoT_pool,
            tc.tile_pool(name="tp2", bufs=2, space="PSUM") as tp2,
            tc.tile_pool(name="pj", bufs=4, space="PSUM") as pj,
        ):
            aoT = [aoT_pool.tile([128, T], MMDT, tag="aoT", name="aoT") for _ in range(CCH)]
            for t in range(TC):
                for c in range(CCH):
                    ps = tp2.tile([128, 128], FP, tag="tp")
                    nc.tensor.transpose(
                        ps[:], attn_out[t][:, c * 128 : (c + 1) * 128], identity[:]
                    )
                    nc.vector.tensor_copy(aoT[c][:, t * 128 : (t + 1) * 128], ps[:])
            for t in range(TC):
                y = y_pool.tile([128, C], FP, tag="y")
                for nh in range(2):
                    ps = pj.tile([128, 384], FP, tag="pj")
                    for c in range(CCH):
                        nc.tensor.matmul(
                            ps[:],
                            aoT[c][:, t * 128 : (t + 1) * 128],
                            wp[c][:, nh * 384 : (nh + 1) * 384],
                            start=(c == 0),
                            stop=(c == CCH - 1),
                        )
                    nc.vector.tensor_add(
                        y[:, nh * 384 : (nh + 1) * 384],
                        ps[:],
                        b_bcast[:, nh * 384 : (nh + 1) * 384],
                    )
                nc.sync.dma_start(outa[t * 128 : (t + 1) * 128, :], y[:])

    nc.finalize()
    return nc


_NC_CACHE = {}


def _get_nc():
    if "nc" not in _NC_CACHE:
        import os

        fast = os.environ.get("KERNEL_FAST", "1") == "1"
        _NC_CACHE["nc"] = build(fast=fast)
    return _NC_CACHE["nc"]


def kernel(x, w_qkv, w_proj, b_proj):
    """Full inputs in, full output out. Shards batch across 8 NeuronCores."""
    assert x.shape == (N_CORES, T, C), x.shape
    nc = _get_nc()
    in_maps = [
        {
            "x": np.ascontiguousarray(x[i], dtype=np.float32),
            "w_qkv": np.ascontiguousarray(w_qkv, dtype=np.float32),
            "w_proj": np.ascontiguousarray(w_proj, dtype=np.float32),
            "b_proj": np.ascontiguousarray(b_proj, dtype=np.float32),
        }
        for i in range(N_CORES)
    ]
    res = run_bass_kernel_spmd(nc, in_maps, list(range(N_CORES)))
    return np.stack([res.results[i]["out"] for i in range(N_CORES)], axis=0)


# revision 21
# speedup vs baseline: 1.1935x; 1.1935x over previous
"""Multi-head attention (B=8, N=1024, C=768, H=12) on 8 Trainium2 NeuronCores.

Strategy: pure data parallelism over the batch dimension — each of the 8
cores computes full attention for one batch element; weights are
replicated. No collectives needed.

Per-core dataflow (all matmuls expressed as out = lhsT.T @ rhs on the PE):
  1. xT  = transpose(x)                          (PE identity-transpose, 48 blocks)
  2. qkT = w_qkv[:, :1536].T @ xT  (q,k feature-major)   lhsT = w_qkv chunks
     v   = x @ w_qkv[:, 1536:]    (v token-major)        lhsT = xT chunks
  3. per head pair (2 heads share a 128-row qkT chunk → row-tiled K=64 matmuls):
       scoresT[m,n] = k_h @ q_h^T   (lhsT = kT slice, rhs = qT slice)
       expT = exp(scale * scoresT)  (ScalarE, softmax max-subtraction skipped:
                                     |scores*scale| < ~2, exp is safe in fp32)
       U[n, 0:65] += expT[mchunk].T @ [v_h | 1]  (ones column gives the softmax
                                                  denominator in U[:, 64])
       attn_out[n, h*64:(h+1)*64] = U[:, :64] * (1/U[:, 64])
  4. attn_outT = transpose(attn_out); y = attn_outT.T @ w_proj + b
"""

import os
import sys

for _p in ("/opt/trn_rl_repo", "/root/.axon_site/_ro/trn_rl_repo"):
    if os.path.isdir(_p) and _p not in sys.path:
        sys.path.append(_p)

from contextlib import ExitStack

import numpy as np

import concourse.bass as bass
import concourse.tile as tile
from concourse import bacc, mybir
from concourse.bass_utils import run_bass_kernel_spmd
from concourse.masks import make_identity

FP = mybir.dt.float32
BF16 = mybir.dt.bfloat16
F32R = mybir.dt.float32r
N_CORES = 8
T = 1024  # tokens per core (batch element)
C = 768
H = 12
D = 64
SCALE = D ** (-0.5)
TC = T // 128  # 8 token chunks
CCH = C // 128  # 6 channel chunks
NPAIR = H // 2  # 6 head pairs

Exp = mybir.ActivationFunctionType.Exp


def build(n_cores: int = N_CORES, fast: bool = True):
    # fast=True: run the large matmuls (qkv / scores / proj, all N>=256) with
    # operands tagged float32r — the PE's full-rate fp32 mode (1 cycle/row vs
    # 4 for plain fp32). Storage is IEEE fp32 either way; only the matmul
    # interpretation changes.
    MMDT = F32R if fast else FP
    nc = bacc.Bacc(
        "TRN2", target_bir_lowering=False, debug=False, num_devices=n_cores
    )
    wdma = nc.gpsimd.dma_start if fast else nc.sync.dma_start
    x = nc.declare_dram_parameter("x", [T, C], FP, isOutput=False)
    w_qkv = nc.declare_dram_parameter("w_qkv", [C, 3 * C], FP, isOutput=False)
    w_proj = nc.declare_dram_parameter("w_proj", [C, C], FP, isOutput=False)
    b_proj = nc.declare_dram_parameter("b_proj", [C], FP, isOutput=False)
    out = nc.declare_dram_parameter("out", [T, C], FP, isOutput=True)

    xa, wqa, wpa, outa = x.ap(), w_qkv.ap(), w_proj.ap(), out.ap()
    ba = b_proj.ap()
    b_bcast_src = bass.AP(tensor=ba.tensor, offset=ba.offset, ap=[[0, 128]] + ba.ap)

    with tile.TileContext(nc) as tc, ExitStack() as ctx:
        # ---- persistent pools (live for the whole kernel) ----
        consts = ctx.enter_context(tc.tile_pool(name="consts", bufs=1))
        qk_pool = ctx.enter_context(tc.tile_pool(name="qk", bufs=12))
        v_pool = ctx.enter_context(tc.tile_pool(name="v65", bufs=TC))
        ao_pool = ctx.enter_context(tc.tile_pool(name="attn_out", bufs=TC))
        wp_pool = ctx.enter_context(tc.tile_pool(name="wp", bufs=CCH))
        y_pool = ctx.enter_context(tc.tile_pool(name="y", bufs=2))
        r_pool = ctx.enter_context(tc.tile_pool(name="r", bufs=4))

        identity = consts.tile([128, 128], FP)
        make_identity(nc, identity)
        ones_h = consts.tile([128, 2 * H], FP)
        nc.vector.memset(ones_h[:], 1.0)

        v65 = [v_pool.tile([128, H, 128], BF16 if fast else FP, tag="v65", name="v65") for _ in range(TC)]
        attn_out = [ao_pool.tile([128, C], FP, tag="ao", name="ao") for _ in range(TC)]
        qkT = [qk_pool.tile([128, T], BF16 if fast else FP, tag="qk", name="qk") for _ in range(12)]

        # ================= phase 1: xT, qkT, v =================
        with (
            tc.tile_pool(name="xstage", bufs=2) as xs_pool,
            tc.tile_pool(name="xT", bufs=CCH) as xT_pool,
            tc.tile_pool(name="mm1", bufs=4, space="PSUM") as mm1,
            tc.tile_pool(name="tp1", bufs=2, space="PSUM") as tp1,
        ):
            xT = [xT_pool.tile([128, T], MMDT, tag="xT", name="xT") for _ in range(CCH)]
            for t in range(TC):
                xs = xs_pool.tile([128, C], FP, tag="xs")
                nc.sync.dma_start(xs[:], xa[t * 128 : (t + 1) * 128, :])
                for c in range(CCH):
                    ps = tp1.tile([128, 128], FP, tag="tp")
                    nc.tensor.transpose(
                        ps[:], xs[:, c * 128 : (c + 1) * 128], identity[:]
                    )
                    nc.any.tensor_copy(xT[c][:, t * 128 : (t + 1) * 128], ps[:])

            # families 1 (qT/kT) and 2 (v) interleaved: f2's groups fill
            # the PE while f1's larger weight DMA streams in
            with (
                tc.tile_pool(name="wq2", bufs=CCH) as wq2_pool,
                tc.tile_pool(name="wq1", bufs=CCH) as wq1_pool,
            ):
                wq2 = []
                for c in range(CCH):
                    w2 = wq2_pool.tile([128, C], MMDT, tag="wq2", name="wq2")
                    wdma(w2[:], wqa[c * 128 : (c + 1) * 128, 2 * C : 3 * C])
                    wq2.append(w2)
                wq1 = []
                for c in range(CCH):
                    w1 = wq1_pool.tile([128, 2 * C], MMDT, tag="wq1", name="wq1")
                    wdma(w1[:], wqa[c * 128 : (c + 1) * 128, 0 : 2 * C])
                    wq1.append(w1)

                def emit_f2(t):
                    nc.vector.memset(v65[t][:, :, D:], 1.0)
                    for nh in range(2):
                        ps = mm1.tile([128, 384], FP, tag="mm", name="mm")
                        for c in range(CCH):
                            nc.tensor.matmul(
                                ps[:],
                                xT[c][:, t * 128 : (t + 1) * 128],
                                wq2[c][:, nh * 384 : (nh + 1) * 384],
                                start=(c == 0),
                                stop=(c == CCH - 1),
                            )
                        nc.any.tensor_copy(
                            v65[t][:, nh * 6 : (nh + 1) * 6, 0:D],
                            ps.rearrange("p (g d) -> p g d", g=6),
                        )

                def emit_f1(j):
                    for nh in range(2):
                        ps = mm1.tile([128, 512], FP, tag="mm", name="mm")
                        for c in range(CCH):
                            nc.tensor.matmul(
                                ps[:],
                                wq1[c][:, j * 128 : (j + 1) * 128],
                                xT[c][:, nh * 512 : (nh + 1) * 512],
                                start=(c == 0),
                                stop=(c == CCH - 1),
                            )
                        nc.vector.tensor_copy(
                            qkT[j][:, nh * 512 : (nh + 1) * 512], ps[:]
                        )

                f1_order = (0, 6, 1, 7, 2, 8, 3, 9, 4, 10, 5, 11)
                for k in range(12):
                    if k < TC:
                        emit_f2(k)
                    emit_f1(f1_order[k])

        # proj weights + bias: only needed in phase 3; issue DMAs after the
        # phase-1 weight loads so they don't starve the first matmuls
        wp = []
        for c in range(CCH):
            w = wp_pool.tile([128, C], MMDT, tag="wp")
            wdma(w[:], wpa[c * 128 : (c + 1) * 128, :])
            wp.append(w)
        b_bcast = consts.tile([128, C], FP)
        nc.sync.dma_start(b_bcast[:], b_bcast_src)

        # ================= phase 2: attention =================
        # expT bufs=4: pair p's scores/exp overlap pair p-1's U^T so the PE
        # never drains while ScalarE works through the exps (HAM stays warm).
        with (
            tc.tile_pool(name="expT", bufs=4) as exp_pool,
            tc.tile_pool(name="uT", bufs=2) as uT_pool,
            tc.tile_pool(name="sc", bufs=2, space="PSUM") as sc_psum,
            tc.tile_pool(name="u", bufs=2, space="PSUM") as u_psum,
            tc.tile_pool(name="tpu", bufs=2, space="PSUM") as tpu_psum,
        ):
            def emit_scores_exp(p, eAB):
                for j in range(TC):  # key-token chunks (m)
                    psAB = [
                        sc_psum.tile([128, T], FP, tag="sc", name="sc")
                        for _ in range(2)
                    ]
                    for half in range(2):
                        base = 64 * half
                        for nh in range(2):
                            nc.tensor.matmul(
                                psAB[half][:, nh * 512 : (nh + 1) * 512],
                                qkT[6 + p][
                                    base : base + 64, j * 128 : (j + 1) * 128
                                ],
                                qkT[p][
                                    base : base + 64, nh * 512 : (nh + 1) * 512
                                ],
                                start=True,
                                stop=True,
                            )
                        nc.scalar.activation(
                            eAB[half][:, j, :], psAB[half][:], Exp, scale=SCALE
                        )

            def emit_u(p, eAB):
                # U^T[d, n] = sum_m v_aug[m, d] expT[m, n]; v stationary, expT
                # moving at N=512. Row 64 = softmax denominator (ones column).
                for half in range(2):
                    h = 2 * p + half
                    e = eAB[half]
                    uT_sb = uT_pool.tile([D + 2, T], FP, tag="uT", name="uT")
                    ups = [
                        u_psum.tile([128, 512], FP, tag="u", name="u")
                        for _ in range(2)
                    ]
                    for j in range(TC):  # lhsT (v) reused across both halves
                        for nh in range(2):
                            nc.tensor.matmul(
                                ups[nh][:],
                                v65[j][:, h, :],
                                e[:, j, nh * 512 : (nh + 1) * 512],
                                start=(j == 0),
                                stop=(j == TC - 1),
                            )
                    for nh in range(2):
                        nc.vector.tensor_copy(
                            uT_sb[:, nh * 512 : (nh + 1) * 512],
                            ups[nh][0 : D + 2, :],
                        )
                    # transpose U^T back to token-major, normalize by 1/row64
                    for i in range(TC):
                        tps = tpu_psum.tile([128, D + 2], FP, tag="tpu", name="tpu")
                        nc.tensor.transpose(
                            tps[:],
                            uT_sb[:, i * 128 : (i + 1) * 128],
                            identity[0 : D + 2, 0 : D + 2],
                        )
                        r = r_pool.tile([128, 1], FP, tag="r")
                        nc.vector.reciprocal(r[:], tps[:, D : D + 1])
                        nc.vector.tensor_scalar_mul(
                            attn_out[i][:, h * D : (h + 1) * D], tps[:, 0:D], r[:]
                        )

            # one-pair-deep software pipeline: emit scores/exp for pair p
            # before U^T of pair p-1 so the PE keeps ScalarE fed across
            # pair boundaries
            eAB_prev = None
            for p in range(NPAIR):
                eAB = [
                    exp_pool.tile(
                        [128, TC, T], BF16 if fast else FP, tag="expT", name="expT"
                    )
                    for _ in range(2)
                ]
                emit_scores_exp(p, eAB)
                if eAB_prev is not None:
                    emit_u(p - 1, eAB_prev)
                eAB_prev = eAB
            emit_u(NPAIR - 1, eAB_prev)

        # ================= phase 3: attn_outT, proj =================
        with (
            tc.tile_pool(name="aoT", bufs=CCH) as aoT_pool,
            tc.tile_pool(name="tp2", bufs=2, space="PSUM") as tp2,
            tc.tile_pool(name="pj", bufs=4, space="PSUM") as pj,
        ):
            aoT = [
                aoT_pool.tile([128, T], MMDT, tag="aoT", name="aoT")
                for _ in range(CCH)
            ]
            for t in range(TC):
                for c in range(CCH):
                    ps = tp2.tile([128, 128], FP, tag="tp")
                    nc.tensor.transpose(
                        ps[:], attn_out[t][:, c * 128 : (c + 1) * 128], identity[:]
                    )
                    nc.any.tensor_copy(aoT[c][:, t * 128 : (t + 1) * 128], ps[:])
            for t in range(TC):
                y = y_pool.tile([128, C], FP, tag="y")
                for nh in range(2):
                    ps = pj.tile([128, 384], FP, tag="pj")
                    for c in range(CCH):
                        nc.tensor.matmul(
                            ps[:],
                            aoT[c][:, t * 128 : (t + 1) * 128],
                            wp[c][:, nh * 384 : (nh + 1) * 384],
                            start=(c == 0),
                            stop=(c == CCH - 1),
                        )
                    nc.vector.tensor_add(
                        y[:, nh * 384 : (nh + 1) * 384],
                        ps[:],
                        b_bcast[:, nh * 384 : (nh + 1) * 384],
                    )
                nc.sync.dma_start(outa[t * 128 : (t + 1) * 128, :], y[:])

    nc.finalize()
    return nc


_NC_CACHE = {}


def _get_nc():
    if "nc" not in _NC_CACHE:
        import os

        fast = os.environ.get("KERNEL_FAST", "1") == "1"
        _NC_CACHE["nc"] = build(fast=fast)
    return _NC_CACHE["nc"]


def kernel(x, w_qkv, w_proj, b_proj):
    """Full inputs in, full output out. Shards batch across 8 NeuronCores."""
    assert x.shape == (N_CORES, T, C), x.shape
    nc = _get_nc()
    in_maps = [
        {
            "x": np.ascontiguousarray(x[i], dtype=np.float32),
            "w_qkv": np.ascontiguousarray(w_qkv, dtype=np.float32),
            "w_proj": np.ascontiguousarray(w_proj, dtype=np.float32),
            "b_proj": np.ascontiguousarray(b_proj, dtype=np.float32),
        }
        for i in range(N_CORES)
    ]
    res = run_bass_kernel_spmd(nc, in_maps, list(range(N_CORES)))
    return np.stack([res.results[i]["out"] for i in range(N_CORES)], axis=0)


# revision 23
# speedup vs baseline: 1.2465x; 1.0444x over previous
"""Multi-head attention (B=8, N=1024, C=768, H=12) on 8 Trainium2 NeuronCores.

Strategy: pure data parallelism over the batch dimension — each of the 8
cores computes full attention for one batch element; weights are
replicated. No collectives needed.

Per-core dataflow (all matmuls expressed as out = lhsT.T @ rhs on the PE):
  1. xT  = transpose(x)                    (PE identity-transpose, 48 blocks)
  2. qkT = w_qkv[:, :1536].T @ xT          (q,k feature-major, bf16)
     v   = x @ w_qkv[:, 1536:]             (v token-major, bf16, padded slots)
  3. per head pair (2 heads share a 128-row qkT chunk -> row-tiled K=64):
       scoresT[m,n] = k_h @ q_h^T          (lhsT = kT slice, rhs = qT slice)
       expT = exp(scale * scoresT)         (ScalarE; max-subtraction skipped:
                                            |scores*scale| < ~2, exp safe)
       U^T[d,n] += v_aug[m,d] expT[m,n]    (v stationary incl ones column ->
                                            row 64 = softmax denominator)
       transpose U^T per 128-token chunk, multiply by 1/row64 (per-partition)
  4. attn_outT = transpose(attn_out); y = attn_outT.T @ w_proj + b

All matmul operands are bf16 (inputs rounded on load / eviction); all
accumulation is fp32 in PSUM. Single flat pool scope; emission weaves the
qkT production, score pairs, and U phases so ScalarE's exp stream starts
while phase-1 matmuls still run.
"""

import os
import sys

for _p in ("/opt/trn_rl_repo", "/root/.axon_site/_ro/trn_rl_repo"):
    if os.path.isdir(_p) and _p not in sys.path:
        sys.path.append(_p)

from contextlib import ExitStack

import numpy as np

import concourse.bass as bass
import concourse.tile as tile
from concourse import bacc, mybir
from concourse.bass_utils import run_bass_kernel_spmd
from concourse.masks import make_identity

FP = mybir.dt.float32
BF16 = mybir.dt.bfloat16
N_CORES = 8
T = 1024  # tokens per core (batch element)
C = 768
H = 12
D = 64
SCALE = D ** (-0.5)
TC = T // 128  # 8 token chunks
CCH = C // 128  # 6 channel chunks
NPAIR = H // 2  # 6 head pairs

Exp = mybir.ActivationFunctionType.Exp


def build(n_cores: int = N_CORES, fast: bool = True):
    MMDT = BF16 if fast else FP
    nc = bacc.Bacc(
        "TRN2", target_bir_lowering=False, debug=False, num_devices=n_cores
    )
    wdma = nc.gpsimd.dma_start if fast else nc.sync.dma_start
    x = nc.declare_dram_parameter("x", [T, C], FP, isOutput=False)
    w_qkv = nc.declare_dram_parameter("w_qkv", [C, 3 * C], FP, isOutput=False)
    w_proj = nc.declare_dram_parameter("w_proj", [C, C], FP, isOutput=False)
    b_proj = nc.declare_dram_parameter("b_proj", [C], FP, isOutput=False)
    out = nc.declare_dram_parameter("out", [T, C], FP, isOutput=True)

    xa, wqa, wpa, outa = x.ap(), w_qkv.ap(), w_proj.ap(), out.ap()
    ba = b_proj.ap()
    b_bcast_src = bass.AP(tensor=ba.tensor, offset=ba.offset, ap=[[0, 128]] + ba.ap)

    with tile.TileContext(nc) as tc, ExitStack() as ctx:
        # ---- one flat scope: no pool-boundary serialization anywhere ----
        consts = ctx.enter_context(tc.tile_pool(name="consts", bufs=1))
        qk_pool = ctx.enter_context(tc.tile_pool(name="qk", bufs=12))
        v_pool = ctx.enter_context(tc.tile_pool(name="v65", bufs=TC))
        ao_pool = ctx.enter_context(tc.tile_pool(name="attn_out", bufs=TC))
        wp_pool = ctx.enter_context(tc.tile_pool(name="wp", bufs=CCH))
        y_pool = ctx.enter_context(tc.tile_pool(name="y", bufs=2))
        r_pool = ctx.enter_context(tc.tile_pool(name="r", bufs=4))
        xs_pool = ctx.enter_context(tc.tile_pool(name="xstage", bufs=2))
        xT_pool = ctx.enter_context(tc.tile_pool(name="xT", bufs=CCH))
        wq1_pool = ctx.enter_context(tc.tile_pool(name="wq1", bufs=CCH))
        wq2_pool = ctx.enter_context(tc.tile_pool(name="wq2", bufs=CCH))
        exp_pool = ctx.enter_context(tc.tile_pool(name="expT", bufs=3))
        uT_pool = ctx.enter_context(tc.tile_pool(name="uT", bufs=2))
        aoT_pool = ctx.enter_context(tc.tile_pool(name="aoT", bufs=CCH))
        # PSUM: sc 2x2 banks + accA 2x1 + accB 2x1 = 8 banks
        sc_psum = ctx.enter_context(tc.tile_pool(name="sc", bufs=2, space="PSUM"))
        accA = ctx.enter_context(tc.tile_pool(name="accA", bufs=2, space="PSUM"))
        accB = ctx.enter_context(tc.tile_pool(name="accB", bufs=2, space="PSUM"))

        identity = consts.tile([128, 128], FP)
        make_identity(nc, identity)

        v65 = [
            v_pool.tile([128, H, 128], MMDT, tag="v65", name="v65")
            for _ in range(TC)
        ]
        attn_out = [
            ao_pool.tile([128, C], FP, tag="ao", name="ao") for _ in range(TC)
        ]
        qkT = [qk_pool.tile([128, T], MMDT, tag="qk", name="qk") for _ in range(12)]
        xT = [xT_pool.tile([128, T], MMDT, tag="xT", name="xT") for _ in range(CCH)]
        aoT = [
            aoT_pool.tile([128, T], MMDT, tag="aoT", name="aoT") for _ in range(CCH)
        ]

        # x load + transpose (48 PE identity-transposes)
        for t in range(TC):
            xs = xs_pool.tile([128, C], FP, tag="xs", name="xs")
            nc.sync.dma_start(xs[:], xa[t * 128 : (t + 1) * 128, :])
            for c in range(CCH):
                ps = accA.tile([128, 512], FP, tag="accA", name="accA")
                nc.tensor.transpose(
                    ps[:, 0:128], xs[:, c * 128 : (c + 1) * 128], identity[:]
                )
                nc.any.tensor_copy(xT[c][:, t * 128 : (t + 1) * 128], ps[:, 0:128])

        # weight loads (casting DMA f32 -> bf16)
        wq2 = []
        for c in range(CCH):
            w2 = wq2_pool.tile([128, C], MMDT, tag="wq2", name="wq2")
            wdma(w2[:], wqa[c * 128 : (c + 1) * 128, 2 * C : 3 * C])
            wq2.append(w2)
        wq1 = []
        for c in range(CCH):
            w1 = wq1_pool.tile([128, 2 * C], MMDT, tag="wq1", name="wq1")
            wdma(w1[:], wqa[c * 128 : (c + 1) * 128, 0 : 2 * C])
            wq1.append(w1)
        wp = []
        for c in range(CCH):
            w3 = wp_pool.tile([128, C], MMDT, tag="wp", name="wp")
            wdma(w3[:], wpa[c * 128 : (c + 1) * 128, :])
            wp.append(w3)
        b_bcast = consts.tile([128, C], FP)
        nc.sync.dma_start(b_bcast[:], b_bcast_src)

        def emit_f1(j):
            # qkT[j] = w_qkv[:, j-chunk].T @ x^T
            for nh in range(2):
                ps = accA.tile([128, 512], FP, tag="accA", name="accA")
                for c in range(CCH):
                    nc.tensor.matmul(
                        ps[:],
                        wq1[c][:, j * 128 : (j + 1) * 128],
                        xT[c][:, nh * 512 : (nh + 1) * 512],
                        start=(c == 0),
                        stop=(c == CCH - 1),
                    )
                nc.any.tensor_copy(qkT[j][:, nh * 512 : (nh + 1) * 512], ps[:])

        def emit_f2(t):
            # v[t] = x[t-chunk] @ w_qkv[:, v-cols]; ones column at slot 64
            nc.vector.memset(v65[t][:, :, D:], 1.0)
            for nh in range(2):
                ps = accA.tile([128, 512], FP, tag="accA", name="accA")
                for c in range(CCH):
                    nc.tensor.matmul(
                        ps[:, 0:384],
                        xT[c][:, t * 128 : (t + 1) * 128],
                        wq2[c][:, nh * 384 : (nh + 1) * 384],
                        start=(c == 0),
                        stop=(c == CCH - 1),
                    )
                nc.any.tensor_copy(
                    v65[t][:, nh * 6 : (nh + 1) * 6, 0:D],
                    ps[:, 0:384].rearrange("p (g d) -> p g d", g=6),
                )

        def emit_scores_exp(p, eAB):
            for j in range(TC):  # key-token chunks (m)
                psAB = [
                    sc_psum.tile([128, T], FP, tag="sc", name="sc")
                    for _ in range(2)
                ]
                for half in range(2):
                    base = 64 * half
                    for nh in range(2):
                        nc.tensor.matmul(
                            psAB[half][:, nh * 512 : (nh + 1) * 512],
                            qkT[6 + p][base : base + 64, j * 128 : (j + 1) * 128],
                            qkT[p][base : base + 64, nh * 512 : (nh + 1) * 512],
                            start=True,
                            stop=True,
                        )
                    nc.scalar.activation(
                        eAB[half][:, j, :], psAB[half][:], Exp, scale=SCALE
                    )

        def emit_u(p, eAB):
            # U^T[d, n] = sum_m v_aug[m, d] expT[m, n]; v stationary, expT
            # moving at N=512. Row 64 = softmax denominator (ones column).
            for half in range(2):
                h = 2 * p + half
                e = eAB[half]
                uT_sb = uT_pool.tile([D + 2, T], FP, tag="uT", name="uT")
                ups = [
                    accB.tile([128, 512], FP, tag="accB", name="accB")
                    for _ in range(2)
                ]
                for j in range(TC):  # lhsT (v) reused across both halves
                    for nh in range(2):
                        nc.tensor.matmul(
                            ups[nh][:],
                            v65[j][:, h, :],
                            e[:, j, nh * 512 : (nh + 1) * 512],
                            start=(j == 0),
                            stop=(j == TC - 1),
                        )
                for nh in range(2):
                    nc.vector.tensor_copy(
                        uT_sb[:, nh * 512 : (nh + 1) * 512], ups[nh][0 : D + 2, :]
                    )
                # transpose U^T back to token-major, normalize by 1/row64
                for i in range(TC):
                    tps = accA.tile([128, 512], FP, tag="accA", name="accA")
                    nc.tensor.transpose(
                        tps[:, 0 : D + 2],
                        uT_sb[:, i * 128 : (i + 1) * 128],
                        identity[0 : D + 2, 0 : D + 2],
                    )
                    r = r_pool.tile([128, 1], FP, tag="r", name="r")
                    nc.vector.reciprocal(r[:], tps[:, D : D + 1])
                    nc.vector.tensor_scalar_mul(
                        attn_out[i][:, h * D : (h + 1) * D], tps[:, 0:D], r[:]
                    )

        def emit_aotp(c):
            # head pair c filled attn_out cols c*128:(c+1)*128 == proj lhsT c
            for t in range(TC):
                ps = accA.tile([128, 512], FP, tag="accA", name="accA")
                nc.tensor.transpose(
                    ps[:, 0:128],
                    attn_out[t][:, c * 128 : (c + 1) * 128],
                    identity[:],
                )
                nc.any.tensor_copy(aoT[c][:, t * 128 : (t + 1) * 128], ps[:, 0:128])

        # ---- woven emission schedule ----
        f1_order = (0, 6, 1, 7, 2, 8, 3, 9, 4, 10, 5, 11)
        emit_f1(0)
        emit_f1(6)
        eAB_list = []

        def new_pair():
            eAB = [
                exp_pool.tile([128, TC, T], MMDT, tag="expT", name="expT")
                for _ in range(2)
            ]
            eAB_list.append(eAB)
            return eAB

        emit_scores_exp(0, new_pair())
        for t in range(TC):
            emit_f2(t)
        emit_f1(1)
        emit_f1(7)
        emit_scores_exp(1, new_pair())
        for p in range(2, NPAIR):
            emit_f1(f1_order[2 * p])
            emit_f1(f1_order[2 * p + 1])
            emit_u(p - 2, eAB_list[p - 2])
            emit_aotp(p - 2)
            emit_scores_exp(p, new_pair())
        for p in (NPAIR - 2, NPAIR - 1):
            emit_u(p, eAB_list[p])
            emit_aotp(p)

        # proj: y = attn_outT.T @ w_proj + b
        for t in range(TC):
            y = y_pool.tile([128, C], FP, tag="y", name="y")
            for nh in range(2):
                ps = accA.tile([128, 512], FP, tag="accA", name="accA")
                for c in range(CCH):
                    nc.tensor.matmul(
                        ps[:, 0:384],
                        aoT[c][:, t * 128 : (t + 1) * 128],
                        wp[c][:, nh * 384 : (nh + 1) * 384],
                        start=(c == 0),
                        stop=(c == CCH - 1),
                    )
                nc.vector.tensor_add(
                    y[:, nh * 384 : (nh + 1) * 384],
                    ps[:, 0:384],
                    b_bcast[:, nh * 384 : (nh + 1) * 384],
                )
            nc.sync.dma_start(outa[t * 128 : (t + 1) * 128, :], y[:])

    nc.finalize()
    return nc


_NC_CACHE = {}


def _get_nc():
    fast = os.environ.get("KERNEL_FAST", "1") == "1"
    if "nc" not in _NC_CACHE:
        _NC_CACHE["nc"] = build(fast=fast)
    return _NC_CACHE["nc"]


def kernel(x, w_qkv, w_proj, b_proj):
    """Full inputs in, full output out. Shards batch across 8 NeuronCores."""
    assert x.shape == (N_CORES, T, C), x.shape
    nc = _get_nc()
    in_maps = [
        {
            "x": np.ascontiguousarray(x[i], dtype=np.float32),
            "w_qkv": np.ascontiguousarray(w_qkv, dtype=np.float32),
            "w_proj": np.ascontiguousarray(w_proj, dtype=np.float32),
            "b_proj": np.ascontiguousarray(b_proj, dtype=np.float32),
        }
        for i in range(N_CORES)
    ]
    res = run_bass_kernel_spmd(nc, in_maps, list(range(N_CORES)))
    return np.stack([res.results[i]["out"] for i in range(N_CORES)], axis=0)


# revision 35
# speedup vs baseline: 1.3374x; 1.0729x over previous
"""Multi-head attention (B=8, N=1024, C=768, H=12) on 8 Trainium2 NeuronCores.

Strategy: pure data parallelism over the batch dimension — each of the 8
cores computes full attention for one batch element; weights are
replicated. No collectives needed.

Per-core dataflow (all matmuls expressed as out = lhsT.T @ rhs on the PE):
  1. xT  = transpose(x)                    (PE identity-transpose, 48 blocks)
  2. qkT = w_qkv[:, :1536].T @ xT          (q,k feature-major, bf16)
     v   = x @ w_qkv[:, 1536:]             (v token-major, bf16, padded slots)
  3. per head pair (2 heads share a 128-row qkT chunk -> row-tiled K=64):
       scoresT[m,n] = k_h @ q_h^T          (lhsT = kT slice, rhs = qT slice)
       expT = exp(scale * scoresT)         (ScalarE; max-subtraction skipped:
                                            |scores*scale| < ~2, exp safe)
       U^T[d,n] += v_aug[m,d] expT[m,n]    (v stationary incl ones column ->
                                            row 64 = softmax denominator)
       transpose U^T per 128-token chunk, multiply by 1/row64 (per-partition)
  4. attn_outT = transpose(attn_out); y = attn_outT.T @ w_proj + b

All matmul operands are bf16 (inputs rounded on load / eviction); all
accumulation is fp32 in PSUM. Single flat pool scope; emission weaves the
qkT production, score pairs, and U phases so ScalarE's exp stream starts
while phase-1 matmuls still run.
"""

import os
import sys

for _p in ("/opt/trn_rl_repo", "/root/.axon_site/_ro/trn_rl_repo"):
    if os.path.isdir(_p) and _p not in sys.path:
        sys.path.append(_p)

from contextlib import ExitStack

import numpy as np

import concourse.bass as bass
import concourse.tile as tile
from concourse import bacc, mybir
from concourse.bass_utils import run_bass_kernel_spmd
from concourse.masks import make_identity

FP = mybir.dt.float32
BF16 = mybir.dt.bfloat16
N_CORES = 8
T = 1024  # tokens per core (batch element)
C = 768
H = 12
D = 64
SCALE = D ** (-0.5)
TC = T // 128  # 8 token chunks
CCH = C // 128  # 6 channel chunks
NPAIR = H // 2  # 6 head pairs

Exp = mybir.ActivationFunctionType.Exp


def build(n_cores: int = N_CORES, fast: bool = True):
    MMDT = BF16 if fast else FP
    nc = bacc.Bacc(
        "TRN2", target_bir_lowering=False, debug=False, num_devices=n_cores
    )
    wdma = nc.gpsimd.dma_start if fast else nc.sync.dma_start
    x = nc.declare_dram_parameter("x", [T, C], FP, isOutput=False)
    w_qkv = nc.declare_dram_parameter("w_qkv", [C, 3 * C], FP, isOutput=False)
    w_proj = nc.declare_dram_parameter("w_proj", [C, C], FP, isOutput=False)
    b_proj = nc.declare_dram_parameter("b_proj", [C], FP, isOutput=False)
    out = nc.declare_dram_parameter("out", [T, C], FP, isOutput=True)

    xa, wqa, wpa, outa = x.ap(), w_qkv.ap(), w_proj.ap(), out.ap()
    ba = b_proj.ap()
    b_bcast_src = bass.AP(tensor=ba.tensor, offset=ba.offset, ap=[[0, 128]] + ba.ap)

    with tile.TileContext(nc) as tc, ExitStack() as ctx:
        # ---- one flat scope: no pool-boundary serialization anywhere ----
        consts = ctx.enter_context(tc.tile_pool(name="consts", bufs=1))
        qk_pool = ctx.enter_context(tc.tile_pool(name="qk", bufs=12))
        v_pool = ctx.enter_context(tc.tile_pool(name="v65", bufs=TC))
        ao_pool = ctx.enter_context(tc.tile_pool(name="attn_out", bufs=TC))
        wp_pool = ctx.enter_context(tc.tile_pool(name="wp", bufs=1))
        y_pool = ctx.enter_context(tc.tile_pool(name="y", bufs=2))
        r_pool = ctx.enter_context(tc.tile_pool(name="r", bufs=4))
        xs_pool = ctx.enter_context(tc.tile_pool(name="xstage", bufs=1))
        xT_pool = ctx.enter_context(tc.tile_pool(name="xT", bufs=CCH))
        wq1_pool = ctx.enter_context(tc.tile_pool(name="wq1", bufs=1))
        wq2_pool = ctx.enter_context(tc.tile_pool(name="wq2", bufs=1))
        exp_pool = ctx.enter_context(tc.tile_pool(name="expT", bufs=3))
        uT_pool = ctx.enter_context(tc.tile_pool(name="uT", bufs=2))
        aoT_pool = ctx.enter_context(tc.tile_pool(name="aoT", bufs=CCH))
        # PSUM: sc 2x2 banks + accA 2x1 + accB 2x1 = 8 banks
        sc_psum = ctx.enter_context(tc.tile_pool(name="sc", bufs=2, space="PSUM"))
        accA = ctx.enter_context(tc.tile_pool(name="accA", bufs=2, space="PSUM"))
        accB = ctx.enter_context(tc.tile_pool(name="accB", bufs=2, space="PSUM"))

        identity = consts.tile([128, 128], FP)
        make_identity(nc, identity)
        identity_h = consts.tile([128, 128], MMDT)
        make_identity(nc, identity_h)

        v65 = [
            v_pool.tile([128, H, 128], MMDT, tag="v65", name="v65")
            for _ in range(TC)
        ]
        attn_out = [
            ao_pool.tile([128, C], MMDT, tag="ao", name="ao") for _ in range(TC)
        ]
        qkT = [qk_pool.tile([128, T], MMDT, tag="qk", name="qk") for _ in range(12)]
        xT = [xT_pool.tile([128, T], MMDT, tag="xT", name="xT") for _ in range(CCH)]
        aoT = [
            aoT_pool.tile([128, T], MMDT, tag="aoT", name="aoT") for _ in range(CCH)
        ]

        # One big casting DMA per input (descriptor fan-out parallelizes
        # across engines; avoids 26 serial SWDGE descriptor generations).
        # Priority: x, then qk weights, then v weights, then proj weights.
        def grouped(src_ap, width, ngrp, col0):
            # view a [ngrp*128, W] dram tensor's width-column slice as
            # [128 partitions, ngrp, width]
            row_step = src_ap.ap[0][0]
            return bass.AP(
                tensor=src_ap.tensor,
                offset=src_ap.offset + col0,
                ap=[[row_step, 128], [128 * row_step, ngrp], [1, width]],
            )

        xs_all = xs_pool.tile([128, TC, C], MMDT, tag="xs", name="xs")
        wdma(xs_all[:], grouped(xa, C, TC, 0))
        wq1_all = wq1_pool.tile([128, CCH, 2 * C], MMDT, tag="wq1", name="wq1")
        wdma(wq1_all[:], grouped(wqa, 2 * C, CCH, 0))
        wq2_all = wq2_pool.tile([128, CCH, C], MMDT, tag="wq2", name="wq2")
        wdma(wq2_all[:], grouped(wqa, C, CCH, 2 * C))
        wp_all = wp_pool.tile([128, CCH, C], MMDT, tag="wp", name="wp")
        wdma(wp_all[:], grouped(wpa, C, CCH, 0))
        b_bcast = consts.tile([128, C], FP)
        nc.sync.dma_start(b_bcast[:], b_bcast_src)
        wq1 = [wq1_all[:, c, :] for c in range(CCH)]
        wq2 = [wq2_all[:, c, :] for c in range(CCH)]
        wp = [wp_all[:, c, :] for c in range(CCH)]

        # x transpose (48 PE identity-transposes)
        for t in range(TC):
            xs = xs_all[:, t, :]
            for c in range(CCH):
                ps = accA.tile([128, 512], FP, tag="accA", name="accA")
                psh = ps[:, 0:256].bitcast(MMDT) if fast else ps[:, 0:128]
                nc.tensor.transpose(
                    psh[:, 0:128], xs[:, c * 128 : (c + 1) * 128], identity_h[:]
                )
                nc.any.tensor_copy(xT[c][:, t * 128 : (t + 1) * 128], psh[:, 0:128])

        def emit_f1(j):
            # qkT[j] = w_qkv[:, j-chunk].T @ x^T
            for nh in range(2):
                ps = accA.tile([128, 512], FP, tag="accA", name="accA")
                for c in range(CCH):
                    nc.tensor.matmul(
                        ps[:],
                        wq1[c][:, j * 128 : (j + 1) * 128],
                        xT[c][:, nh * 512 : (nh + 1) * 512],
                        start=(c == 0),
                        stop=(c == CCH - 1),
                    )
                nc.any.tensor_copy(qkT[j][:, nh * 512 : (nh + 1) * 512], ps[:])

        def emit_f2(t):
            # v[t] = x[t-chunk] @ w_qkv[:, v-cols]; ones column at slot 64
            nc.vector.memset(v65[t][:, :, D:], 1.0)
            for nh in range(2):
                ps = accA.tile([128, 512], FP, tag="accA", name="accA")
                for c in range(CCH):
                    nc.tensor.matmul(
                        ps[:, 0:384],
                        xT[c][:, t * 128 : (t + 1) * 128],
                        wq2[c][:, nh * 384 : (nh + 1) * 384],
                        start=(c == 0),
                        stop=(c == CCH - 1),
                    )
                nc.any.tensor_copy(
                    v65[t][:, nh * 6 : (nh + 1) * 6, 0:D],
                    ps[:, 0:384].rearrange("p (g d) -> p g d", g=6),
                )

        def emit_scores_exp(p, eAB):
            for j in range(TC):  # key-token chunks (m)
                psAB = [
                    sc_psum.tile([128, T], FP, tag="sc", name="sc")
                    for _ in range(2)
                ]
                for half in range(2):
                    base = 64 * half
                    for nh in range(2):
                        nc.tensor.matmul(
                            psAB[half][:, nh * 512 : (nh + 1) * 512],
                            qkT[6 + p][base : base + 64, j * 128 : (j + 1) * 128],
                            qkT[p][base : base + 64, nh * 512 : (nh + 1) * 512],
                            start=True,
                            stop=True,
                        )
                    nc.scalar.activation(
                        eAB[half][:, j, :], psAB[half][:], Exp, scale=SCALE
                    )

        def emit_u(p, eAB):
            # U^T[d, n] = sum_m v_aug[m, d] expT[m, n]; v stationary, expT
            # moving at N=512. Row 64 = softmax denominator (ones column).
            for half in range(2):
                h = 2 * p + half
                e = eAB[half]
                uT_sb = uT_pool.tile([D + 2, T], MMDT, tag="uT", name="uT")
                ups = [
                    accB.tile([128, 512], FP, tag="accB", name="accB")
                    for _ in range(2)
                ]
                for j in range(TC):  # lhsT (v) reused across both halves
                    for nh in range(2):
                        nc.tensor.matmul(
                            ups[nh][:],
                            v65[j][:, h, :],
                            e[:, j, nh * 512 : (nh + 1) * 512],
                            start=(j == 0),
                            stop=(j == TC - 1),
                        )
                for nh in range(2):
                    nc.vector.tensor_copy(
                        uT_sb[:, nh * 512 : (nh + 1) * 512], ups[nh][0 : D + 2, :]
                    )
                # transpose U^T back to token-major, normalize by 1/row64
                for i in range(TC):
                    tps = accA.tile([128, 512], FP, tag="accA", name="accA")
                    tph = tps[:, 0:256].bitcast(MMDT) if fast else tps[:, 0 : D + 2]
                    nc.tensor.transpose(
                        tph[:, 0 : D + 2],
                        uT_sb[:, i * 128 : (i + 1) * 128],
                        identity_h[0 : D + 2, 0 : D + 2],
                    )
                    r = r_pool.tile([128, 1], FP, tag="r", name="r")
                    nc.vector.reciprocal(r[:], tph[:, D : D + 1])
                    nc.vector.tensor_scalar_mul(
                        attn_out[i][:, h * D : (h + 1) * D], tph[:, 0:D], r[:]
                    )

        def emit_aotp(c):
            # head pair c filled attn_out cols c*128:(c+1)*128 == proj lhsT c
            for t in range(TC):
                ps = accA.tile([128, 512], FP, tag="accA", name="accA")
                psh = ps[:, 0:256].bitcast(MMDT) if fast else ps[:, 0:128]
                nc.tensor.transpose(
                    psh[:, 0:128],
                    attn_out[t][:, c * 128 : (c + 1) * 128],
                    identity_h[:],
                )
                nc.any.tensor_copy(aoT[c][:, t * 128 : (t + 1) * 128], psh[:, 0:128])

        def emit_proj():
            # proj: y = attn_outT.T @ w_proj + b
            for t in range(TC):
                y = y_pool.tile([128, C], FP, tag="y", name="y")
                for nh in range(2):
                    ps = accA.tile([128, 512], FP, tag="accA", name="accA")
                    for c in range(CCH):
                        nc.tensor.matmul(
                            ps[:, 0:384],
                            aoT[c][:, t * 128 : (t + 1) * 128],
                            wp[c][:, nh * 384 : (nh + 1) * 384],
                            start=(c == 0),
                            stop=(c == CCH - 1),
                        )
                    nc.vector.tensor_add(
                        y[:, nh * 384 : (nh + 1) * 384],
                        ps[:, 0:384],
                        b_bcast[:, nh * 384 : (nh + 1) * 384],
                    )
                nc.sync.dma_start(outa[t * 128 : (t + 1) * 128, :], y[:])

        # ---- woven emission schedule ----
        f1_order = (0, 6, 1, 7, 2, 8, 3, 9, 4, 10, 5, 11)
        emit_f1(0)
        emit_f1(6)
        eAB_list = []

        def new_pair():
            eAB = [
                exp_pool.tile([128, TC, T], MMDT, tag="expT", name="expT")
                for _ in range(2)
            ]
            eAB_list.append(eAB)
            return eAB

        emit_scores_exp(0, new_pair())
        for t in range(TC):
            emit_f2(t)
        emit_f1(1)
        emit_f1(7)
        emit_scores_exp(1, new_pair())
        for p in range(2, NPAIR):
            emit_f1(f1_order[2 * p])
            emit_f1(f1_order[2 * p + 1])
            emit_u(p - 2, eAB_list[p - 2])
            emit_aotp(p - 2)
            emit_scores_exp(p, new_pair())
        emit_u(NPAIR - 2, eAB_list[NPAIR - 2])
        emit_aotp(NPAIR - 2)
        emit_u(NPAIR - 1, eAB_list[NPAIR - 1])
        emit_aotp(NPAIR - 1)

    nc.finalize()
    return nc


_NC_CACHE = {}


def _get_nc():
    fast = os.environ.get("KERNEL_FAST", "1") == "1"
    if "nc" not in _NC_CACHE:
        _NC_CACHE["nc"] = build(fast=fast)
    return _NC_CACHE["nc"]


def kernel(x, w_qkv, w_proj, b_proj):
    """Full inputs in, full output out. Shards batch across 8 NeuronCores."""
    assert x.shape == (N_CORES, T, C), x.shape
    nc = _get_nc()
    in_maps = [
        {
            "x": np.ascontiguousarray(x[i], dtype=np.float32),
            "w_qkv": np.ascontiguousarray(w_qkv, dtype=np.float32),
            "w_proj": np.ascontiguousarray(w_proj, dtype=np.float32),
            "b_proj": np.ascontiguousarray(b_proj, dtype=np.float32),
        }
        for i in range(N_CORES)
    ]
    res = run_bass_kernel_spmd(nc, in_maps, list(range(N_CORES)))
    return np.stack([res.results[i]["out"] for i in range(N_CORES)], axis=0)


# revision 36
# speedup vs baseline: 1.3492x; 1.0088x over previous
"""Multi-head attention (B=8, N=1024, C=768, H=12) on 8 Trainium2 NeuronCores.

Strategy: pure data parallelism over the batch dimension — each of the 8
cores computes full attention for one batch element; weights are
replicated. No collectives needed.

Per-core dataflow (all matmuls expressed as out = lhsT.T @ rhs on the PE):
  1. xT  = transpose(x)                    (PE identity-transpose, 48 blocks)
  2. qkT = w_qkv[:, :1536].T @ xT          (q,k feature-major, bf16)
     v   = x @ w_qkv[:, 1536:]             (v token-major, bf16, padded slots)
  3. per head pair (2 heads share a 128-row qkT chunk -> row-tiled K=64):
       scoresT[m,n] = k_h @ q_h^T          (lhsT = kT slice, rhs = qT slice)
       expT = exp(scale * scoresT)         (ScalarE; max-subtraction skipped:
                                            |scores*scale| < ~2, exp safe)
       U^T[d,n] += v_aug[m,d] expT[m,n]    (v stationary incl ones column ->
                                            row 64 = softmax denominator)
       transpose U^T per 128-token chunk, multiply by 1/row64 (per-partition)
  4. attn_outT = transpose(attn_out); y = attn_outT.T @ w_proj + b

All matmul operands are bf16 (inputs rounded on load / eviction); all
accumulation is fp32 in PSUM. Single flat pool scope; emission weaves the
qkT production, score pairs, and U phases so ScalarE's exp stream starts
while phase-1 matmuls still run.
"""

import os
import sys

for _p in ("/opt/trn_rl_repo", "/root/.axon_site/_ro/trn_rl_repo"):
    if os.path.isdir(_p) and _p not in sys.path:
        sys.path.append(_p)

from contextlib import ExitStack

import numpy as np

import concourse.bass as bass
import concourse.tile as tile
from concourse import bacc, mybir
from concourse.bass_utils import run_bass_kernel_spmd
from concourse.masks import make_identity

FP = mybir.dt.float32
BF16 = mybir.dt.bfloat16
N_CORES = 8
T = 1024  # tokens per core (batch element)
C = 768
H = 12
D = 64
SCALE = D ** (-0.5)
TC = T // 128  # 8 token chunks
CCH = C // 128  # 6 channel chunks
NPAIR = H // 2  # 6 head pairs

Exp = mybir.ActivationFunctionType.Exp


def build(n_cores: int = N_CORES, fast: bool = True):
    MMDT = BF16 if fast else FP
    nc = bacc.Bacc(
        "TRN2", target_bir_lowering=False, debug=False, num_devices=n_cores
    )
    wdma = nc.gpsimd.dma_start if fast else nc.sync.dma_start
    x = nc.declare_dram_parameter("x", [T, C], FP, isOutput=False)
    w_qkv = nc.declare_dram_parameter("w_qkv", [C, 3 * C], FP, isOutput=False)
    w_proj = nc.declare_dram_parameter("w_proj", [C, C], FP, isOutput=False)
    b_proj = nc.declare_dram_parameter("b_proj", [C], FP, isOutput=False)
    out = nc.declare_dram_parameter("out", [T, C], FP, isOutput=True)

    xa, wqa, wpa, outa = x.ap(), w_qkv.ap(), w_proj.ap(), out.ap()
    ba = b_proj.ap()
    b_bcast_src = bass.AP(tensor=ba.tensor, offset=ba.offset, ap=[[0, 128]] + ba.ap)

    with tile.TileContext(nc) as tc, ExitStack() as ctx:
        # ---- one flat scope: no pool-boundary serialization anywhere ----
        consts = ctx.enter_context(tc.tile_pool(name="consts", bufs=1))
        qk_pool = ctx.enter_context(tc.tile_pool(name="qk", bufs=12))
        v_pool = ctx.enter_context(tc.tile_pool(name="v65", bufs=TC))
        ao_pool = ctx.enter_context(tc.tile_pool(name="attn_out", bufs=TC))
        wp_pool = ctx.enter_context(tc.tile_pool(name="wp", bufs=1))
        y_pool = ctx.enter_context(tc.tile_pool(name="y", bufs=2))
        r_pool = ctx.enter_context(tc.tile_pool(name="r", bufs=4))
        xs_pool = ctx.enter_context(tc.tile_pool(name="xstage", bufs=1))
        xT_pool = ctx.enter_context(tc.tile_pool(name="xT", bufs=CCH))
        wq1_pool = ctx.enter_context(tc.tile_pool(name="wq1", bufs=1))
        wq2_pool = ctx.enter_context(tc.tile_pool(name="wq2", bufs=1))
        exp_pool = ctx.enter_context(tc.tile_pool(name="expT", bufs=3))
        uT_pool = ctx.enter_context(tc.tile_pool(name="uT", bufs=2))
        aoT_pool = ctx.enter_context(tc.tile_pool(name="aoT", bufs=CCH))
        # PSUM: sc 2x2 banks + accA 2x1 + accB 2x1 = 8 banks
        sc_psum = ctx.enter_context(tc.tile_pool(name="sc", bufs=2, space="PSUM"))
        accA = ctx.enter_context(tc.tile_pool(name="accA", bufs=2, space="PSUM"))
        accB = ctx.enter_context(tc.tile_pool(name="accB", bufs=2, space="PSUM"))

        identity = consts.tile([128, 128], FP)
        make_identity(nc, identity)
        identity_h = consts.tile([128, 128], MMDT)
        make_identity(nc, identity_h)

        v65 = [
            v_pool.tile([128, H, 128], MMDT, tag="v65", name="v65")
            for _ in range(TC)
        ]
        attn_out = [
            ao_pool.tile([128, C], MMDT, tag="ao", name="ao") for _ in range(TC)
        ]
        qkT = [qk_pool.tile([128, T], MMDT, tag="qk", name="qk") for _ in range(12)]
        xT = [xT_pool.tile([128, T], MMDT, tag="xT", name="xT") for _ in range(CCH)]
        aoT = [
            aoT_pool.tile([128, T], MMDT, tag="aoT", name="aoT") for _ in range(CCH)
        ]

        # One big casting DMA per input (descriptor fan-out parallelizes
        # across engines; avoids 26 serial SWDGE descriptor generations).
        # Priority: x, then qk weights, then v weights, then proj weights.
        def grouped(src_ap, width, ngrp, col0):
            # view a [ngrp*128, W] dram tensor's width-column slice as
            # [128 partitions, ngrp, width]
            row_step = src_ap.ap[0][0]
            return bass.AP(
                tensor=src_ap.tensor,
                offset=src_ap.offset + col0,
                ap=[[row_step, 128], [128 * row_step, ngrp], [1, width]],
            )

        xs_all = xs_pool.tile([128, TC, C], MMDT, tag="xs", name="xs")
        wdma(xs_all[:], grouped(xa, C, TC, 0))
        wq1_all = wq1_pool.tile([128, CCH, 2 * C], MMDT, tag="wq1", name="wq1")
        wdma(wq1_all[:], grouped(wqa, 2 * C, CCH, 0))
        wq2_all = wq2_pool.tile([128, CCH, C], MMDT, tag="wq2", name="wq2")
        wdma(wq2_all[:], grouped(wqa, C, CCH, 2 * C))
        wp_all = wp_pool.tile([128, CCH, C], MMDT, tag="wp", name="wp")
        wdma(wp_all[:], grouped(wpa, C, CCH, 0))
        b_bcast = consts.tile([128, C], FP)
        nc.sync.dma_start(b_bcast[:], b_bcast_src)
        wq1 = [wq1_all[:, c, :] for c in range(CCH)]
        wq2 = [wq2_all[:, c, :] for c in range(CCH)]
        wp = [wp_all[:, c, :] for c in range(CCH)]

        # x transpose (48 PE identity-transposes)
        for t in range(TC):
            xs = xs_all[:, t, :]
            for c in range(CCH):
                ps = accA.tile([128, 512], FP, tag="accA", name="accA")
                psh = ps[:, 0:256].bitcast(MMDT) if fast else ps[:, 0:128]
                nc.tensor.transpose(
                    psh[:, 0:128], xs[:, c * 128 : (c + 1) * 128], identity_h[:]
                )
                nc.any.tensor_copy(xT[c][:, t * 128 : (t + 1) * 128], psh[:, 0:128])

        def emit_f1(j):
            # qkT[j] = w_qkv[:, j-chunk].T @ x^T
            for nh in range(2):
                ps = accA.tile([128, 512], FP, tag="accA", name="accA")
                for c in range(CCH):
                    nc.tensor.matmul(
                        ps[:],
                        wq1[c][:, j * 128 : (j + 1) * 128],
                        xT[c][:, nh * 512 : (nh + 1) * 512],
                        start=(c == 0),
                        stop=(c == CCH - 1),
                    )
                nc.any.tensor_copy(qkT[j][:, nh * 512 : (nh + 1) * 512], ps[:])

        def emit_f2(t):
            # v[t] = x[t-chunk] @ w_qkv[:, v-cols]; ones column at slot 64
            nc.vector.memset(v65[t][:, :, D:], 1.0)
            for nh in range(2):
                ps = accA.tile([128, 512], FP, tag="accA", name="accA")
                for c in range(CCH):
                    nc.tensor.matmul(
                        ps[:, 0:384],
                        xT[c][:, t * 128 : (t + 1) * 128],
                        wq2[c][:, nh * 384 : (nh + 1) * 384],
                        start=(c == 0),
                        stop=(c == CCH - 1),
                    )
                nc.any.tensor_copy(
                    v65[t][:, nh * 6 : (nh + 1) * 6, 0:D],
                    ps[:, 0:384].rearrange("p (g d) -> p g d", g=6),
                )

        def emit_scores_exp(p, eAB):
            for j in range(TC):  # key-token chunks (m)
                psAB = [
                    sc_psum.tile([128, T], FP, tag="sc", name="sc")
                    for _ in range(2)
                ]
                for half in range(2):
                    base = 64 * half
                    for nh in range(2):
                        nc.tensor.matmul(
                            psAB[half][:, nh * 512 : (nh + 1) * 512],
                            qkT[6 + p][base : base + 64, j * 128 : (j + 1) * 128],
                            qkT[p][base : base + 64, nh * 512 : (nh + 1) * 512],
                            start=True,
                            stop=True,
                        )
                    nc.scalar.activation(
                        eAB[half][:, j, :], psAB[half][:], Exp, scale=SCALE
                    )

        def emit_u(p, eAB):
            # U^T[d, n] = sum_m v_aug[m, d] expT[m, n]; v stationary, expT
            # moving at N=512. Row 64 = softmax denominator (ones column).
            for half in range(2):
                h = 2 * p + half
                e = eAB[half]
                uT_sb = uT_pool.tile([D + 2, T], MMDT, tag="uT", name="uT")
                ups = [
                    accB.tile([128, 512], FP, tag="accB", name="accB")
                    for _ in range(2)
                ]
                for j in range(TC):  # lhsT (v) reused across both halves
                    for nh in range(2):
                        nc.tensor.matmul(
                            ups[nh][:],
                            v65[j][:, h, :],
                            e[:, j, nh * 512 : (nh + 1) * 512],
                            start=(j == 0),
                            stop=(j == TC - 1),
                        )
                for nh in range(2):
                    nc.vector.tensor_copy(
                        uT_sb[:, nh * 512 : (nh + 1) * 512], ups[nh][0 : D + 2, :]
                    )
                # transpose U^T back to token-major, normalize by 1/row64
                for i in range(TC):
                    tps = accA.tile([128, 512], FP, tag="accA", name="accA")
                    tph = tps[:, 0:256].bitcast(MMDT) if fast else tps[:, 0 : D + 2]
                    nc.tensor.transpose(
                        tph[:, 0 : D + 2],
                        uT_sb[:, i * 128 : (i + 1) * 128],
                        identity_h[0 : D + 2, 0 : D + 2],
                    )
                    r = r_pool.tile([128, 1], FP, tag="r", name="r")
                    nc.vector.reciprocal(r[:], tph[:, D : D + 1])
                    nc.vector.tensor_scalar_mul(
                        attn_out[i][:, h * D : (h + 1) * D], tph[:, 0:D], r[:]
                    )

        def emit_aotp(c):
            # head pair c filled attn_out cols c*128:(c+1)*128 == proj lhsT c
            for t in range(TC):
                ps = accA.tile([128, 512], FP, tag="accA", name="accA")
                psh = ps[:, 0:256].bitcast(MMDT) if fast else ps[:, 0:128]
                nc.tensor.transpose(
                    psh[:, 0:128],
                    attn_out[t][:, c * 128 : (c + 1) * 128],
                    identity_h[:],
                )
                nc.any.tensor_copy(aoT[c][:, t * 128 : (t + 1) * 128], psh[:, 0:128])

        def emit_proj():
            # proj: y = attn_outT.T @ w_proj + b
            for t in range(TC):
                y = y_pool.tile([128, C], FP, tag="y", name="y")
                for nh in range(2):
                    ps = accA.tile([128, 512], FP, tag="accA", name="accA")
                    for c in range(CCH):
                        nc.tensor.matmul(
                            ps[:, 0:384],
                            aoT[c][:, t * 128 : (t + 1) * 128],
                            wp[c][:, nh * 384 : (nh + 1) * 384],
                            start=(c == 0),
                            stop=(c == CCH - 1),
                        )
                    nc.vector.tensor_add(
                        y[:, nh * 384 : (nh + 1) * 384],
                        ps[:, 0:384],
                        b_bcast[:, nh * 384 : (nh + 1) * 384],
                    )
                nc.sync.dma_start(outa[t * 128 : (t + 1) * 128, :], y[:])

        # ---- woven emission schedule ----
        f1_order = (0, 6, 1, 7, 2, 8, 3, 9, 4, 10, 5, 11)
        emit_f1(0)
        emit_f1(6)
        eAB_list = []

        def new_pair():
            eAB = [
                exp_pool.tile([128, TC, T], MMDT, tag="expT", name="expT")
                for _ in range(2)
            ]
            eAB_list.append(eAB)
            return eAB

        emit_scores_exp(0, new_pair())
        for t in range(TC):
            emit_f2(t)
        emit_f1(1)
        emit_f1(7)
        emit_scores_exp(1, new_pair())
        for p in range(2, NPAIR):
            emit_f1(f1_order[2 * p])
            emit_f1(f1_order[2 * p + 1])
            emit_scores_exp(p, new_pair())
            emit_u(p - 2, eAB_list[p - 2])
            emit_aotp(p - 2)
        emit_u(NPAIR - 2, eAB_list[NPAIR - 2])
        emit_aotp(NPAIR - 2)
        emit_u(NPAIR - 1, eAB_list[NPAIR - 1])
        emit_aotp(NPAIR - 1)

    nc.finalize()
    return nc


_NC_CACHE = {}


def _get_nc():
    fast = os.environ.get("KERNEL_FAST", "1") == "1"
    if "nc" not in _NC_CACHE:
        _NC_CACHE["nc"] = build(fast=fast)
    return _NC_CACHE["nc"]


def kernel(x, w_qkv, w_proj, b_proj):
    """Full inputs in, full output out. Shards batch across 8 NeuronCores."""
    assert x.shape == (N_CORES, T, C), x.shape
    nc = _get_nc()
    in_maps = [
        {
            "x": np.ascontiguousarray(x[i], dtype=np.float32),
            "w_qkv": np.ascontiguousarray(w_qkv, dtype=np.float32),
            "w_proj": np.ascontiguousarray(w_proj, dtype=np.float32),
            "b_proj": np.ascontiguousarray(b_proj, dtype=np.float32),
        }
        for i in range(N_CORES)
    ]
    res = run_bass_kernel_spmd(nc, in_maps, list(range(N_CORES)))
    return np.stack([res.results[i]["out"] for i in range(N_CORES)], axis=0)


# revision 38
# speedup vs baseline: 1.3596x; 1.0077x over previous
"""Multi-head attention (B=8, N=1024, C=768, H=12) on 8 Trainium2 NeuronCores.

Strategy: pure data parallelism over the batch dimension — each of the 8
cores computes full attention for one batch element; weights are
replicated. No collectives needed.

Per-core dataflow (all matmuls expressed as out = lhsT.T @ rhs on the PE):
  1. xT  = transpose(x)                    (PE identity-transpose, 48 blocks)
  2. qkT = w_qkv[:, :1536].T @ xT          (q,k feature-major, bf16)
     v   = x @ w_qkv[:, 1536:]             (v token-major, bf16, padded slots)
  3. per head pair (2 heads share a 128-row qkT chunk -> row-tiled K=64):
       scoresT[m,n] = k_h @ q_h^T          (lhsT = kT slice, rhs = qT slice)
       expT = exp(scale * scoresT)         (ScalarE; max-subtraction skipped:
                                            |scores*scale| < ~2, exp safe)
       U^T[d,n] += v_aug[m,d] expT[m,n]    (v stationary incl ones column ->
                                            row 64 = softmax denominator)
       transpose U^T per 128-token chunk, multiply by 1/row64 (per-partition)
  4. attn_outT = transpose(attn_out); y = attn_outT.T @ w_proj + b

All matmul operands are bf16 (inputs rounded on load / eviction); all
accumulation is fp32 in PSUM. Single flat pool scope; emission weaves the
qkT production, score pairs, and U phases so ScalarE's exp stream starts
while phase-1 matmuls still run.
"""

import os
import sys

for _p in ("/opt/trn_rl_repo", "/root/.axon_site/_ro/trn_rl_repo"):
    if os.path.isdir(_p) and _p not in sys.path:
        sys.path.append(_p)

from contextlib import ExitStack

import numpy as np

import concourse.bass as bass
import concourse.tile as tile
from concourse import bacc, mybir
from concourse.bass_utils import run_bass_kernel_spmd
from concourse.masks import make_identity

FP = mybir.dt.float32
BF16 = mybir.dt.bfloat16
N_CORES = 8
T = 1024  # tokens per core (batch element)
C = 768
H = 12
D = 64
SCALE = D ** (-0.5)
TC = T // 128  # 8 token chunks
CCH = C // 128  # 6 channel chunks
NPAIR = H // 2  # 6 head pairs

Exp = mybir.ActivationFunctionType.Exp


def build(n_cores: int = N_CORES, fast: bool = True):
    MMDT = BF16 if fast else FP
    nc = bacc.Bacc(
        "TRN2", target_bir_lowering=False, debug=False, num_devices=n_cores
    )
    wdma = nc.gpsimd.dma_start if fast else nc.sync.dma_start
    x = nc.declare_dram_parameter("x", [T, C], FP, isOutput=False)
    w_qkv = nc.declare_dram_parameter("w_qkv", [C, 3 * C], FP, isOutput=False)
    w_proj = nc.declare_dram_parameter("w_proj", [C, C], FP, isOutput=False)
    b_proj = nc.declare_dram_parameter("b_proj", [C], FP, isOutput=False)
    out = nc.declare_dram_parameter("out", [T, C], FP, isOutput=True)

    xa, wqa, wpa, outa = x.ap(), w_qkv.ap(), w_proj.ap(), out.ap()
    ba = b_proj.ap()
    b_bcast_src = bass.AP(tensor=ba.tensor, offset=ba.offset, ap=[[0, 128]] + ba.ap)

    with tile.TileContext(nc) as tc, ExitStack() as ctx:
        # ---- one flat scope: no pool-boundary serialization anywhere ----
        consts = ctx.enter_context(tc.tile_pool(name="consts", bufs=1))
        qk_pool = ctx.enter_context(tc.tile_pool(name="qk", bufs=12))
        v_pool = ctx.enter_context(tc.tile_pool(name="v65", bufs=TC))
        ao_pool = ctx.enter_context(tc.tile_pool(name="attn_out", bufs=TC))
        wp_pool = ctx.enter_context(tc.tile_pool(name="wp", bufs=1))
        y_pool = ctx.enter_context(tc.tile_pool(name="y", bufs=2))
        r_pool = ctx.enter_context(tc.tile_pool(name="r", bufs=4))
        xs_pool = ctx.enter_context(tc.tile_pool(name="xstage", bufs=1))
        xT_pool = ctx.enter_context(tc.tile_pool(name="xT", bufs=CCH))
        wq1_pool = ctx.enter_context(tc.tile_pool(name="wq1", bufs=1))
        wq2_pool = ctx.enter_context(tc.tile_pool(name="wq2", bufs=1))
        exp_pool = ctx.enter_context(tc.tile_pool(name="expT", bufs=4))
        uT_pool = ctx.enter_context(tc.tile_pool(name="uT", bufs=2))
        aoT_pool = ctx.enter_context(tc.tile_pool(name="aoT", bufs=CCH))
        # PSUM: sc 2x2 banks + accA 2x1 + accB 2x1 = 8 banks
        sc_psum = ctx.enter_context(tc.tile_pool(name="sc", bufs=2, space="PSUM"))
        accA = ctx.enter_context(tc.tile_pool(name="accA", bufs=2, space="PSUM"))
        accB = ctx.enter_context(tc.tile_pool(name="accB", bufs=2, space="PSUM"))

        identity = consts.tile([128, 128], FP)
        make_identity(nc, identity)
        identity_h = consts.tile([128, 128], MMDT)
        make_identity(nc, identity_h)

        v65 = [
            v_pool.tile([128, H, 96], MMDT, tag="v65", name="v65")
            for _ in range(TC)
        ]
        attn_out = [
            ao_pool.tile([128, C], MMDT, tag="ao", name="ao") for _ in range(TC)
        ]
        qkT = [qk_pool.tile([128, T], MMDT, tag="qk", name="qk") for _ in range(12)]
        xT = [xT_pool.tile([128, T], MMDT, tag="xT", name="xT") for _ in range(CCH)]
        aoT = [
            aoT_pool.tile([128, T], MMDT, tag="aoT", name="aoT") for _ in range(CCH)
        ]

        # One big casting DMA per input (descriptor fan-out parallelizes
        # across engines; avoids 26 serial SWDGE descriptor generations).
        # Priority: x, then qk weights, then v weights, then proj weights.
        def grouped(src_ap, width, ngrp, col0):
            # view a [ngrp*128, W] dram tensor's width-column slice as
            # [128 partitions, ngrp, width]
            row_step = src_ap.ap[0][0]
            return bass.AP(
                tensor=src_ap.tensor,
                offset=src_ap.offset + col0,
                ap=[[row_step, 128], [128 * row_step, ngrp], [1, width]],
            )

        xs_all = xs_pool.tile([128, TC, C], MMDT, tag="xs", name="xs")
        wdma(xs_all[:], grouped(xa, C, TC, 0))
        wq1_all = wq1_pool.tile([128, CCH, 2 * C], MMDT, tag="wq1", name="wq1")
        wdma(wq1_all[:], grouped(wqa, 2 * C, CCH, 0))
        wq2_all = wq2_pool.tile([128, CCH, C], MMDT, tag="wq2", name="wq2")
        wdma(wq2_all[:], grouped(wqa, C, CCH, 2 * C))
        wp_all = wp_pool.tile([128, CCH, C], MMDT, tag="wp", name="wp")
        wdma(wp_all[:], grouped(wpa, C, CCH, 0))
        b_bcast = consts.tile([128, C], FP)
        nc.sync.dma_start(b_bcast[:], b_bcast_src)
        wq1 = [wq1_all[:, c, :] for c in range(CCH)]
        wq2 = [wq2_all[:, c, :] for c in range(CCH)]
        wp = [wp_all[:, c, :] for c in range(CCH)]

        # x transpose (48 PE identity-transposes)
        for t in range(TC):
            xs = xs_all[:, t, :]
            for c in range(CCH):
                ps = accA.tile([128, 512], FP, tag="accA", name="accA")
                psh = ps[:, 0:256].bitcast(MMDT) if fast else ps[:, 0:128]
                nc.tensor.transpose(
                    psh[:, 0:128], xs[:, c * 128 : (c + 1) * 128], identity_h[:]
                )
                nc.any.tensor_copy(xT[c][:, t * 128 : (t + 1) * 128], psh[:, 0:128])

        def emit_f1(j):
            # qkT[j] = w_qkv[:, j-chunk].T @ x^T
            for nh in range(2):
                ps = accA.tile([128, 512], FP, tag="accA", name="accA")
                for c in range(CCH):
                    nc.tensor.matmul(
                        ps[:],
                        wq1[c][:, j * 128 : (j + 1) * 128],
                        xT[c][:, nh * 512 : (nh + 1) * 512],
                        start=(c == 0),
                        stop=(c == CCH - 1),
                    )
                nc.any.tensor_copy(qkT[j][:, nh * 512 : (nh + 1) * 512], ps[:])

        def emit_f2(t):
            # v[t] = x[t-chunk] @ w_qkv[:, v-cols]; ones column at slot 64
            nc.vector.memset(v65[t][:, :, D:], 1.0)
            for nh in range(2):
                ps = accA.tile([128, 512], FP, tag="accA", name="accA")
                for c in range(CCH):
                    nc.tensor.matmul(
                        ps[:, 0:384],
                        xT[c][:, t * 128 : (t + 1) * 128],
                        wq2[c][:, nh * 384 : (nh + 1) * 384],
                        start=(c == 0),
                        stop=(c == CCH - 1),
                    )
                nc.any.tensor_copy(
                    v65[t][:, nh * 6 : (nh + 1) * 6, 0:D],
                    ps[:, 0:384].rearrange("p (g d) -> p g d", g=6),
                )

        def emit_scores_exp(p, eAB):
            for j in range(TC):  # key-token chunks (m)
                psAB = [
                    sc_psum.tile([128, T], FP, tag="sc", name="sc")
                    for _ in range(2)
                ]
                for half in range(2):
                    base = 64 * half
                    for nh in range(2):
                        nc.tensor.matmul(
                            psAB[half][:, nh * 512 : (nh + 1) * 512],
                            qkT[6 + p][base : base + 64, j * 128 : (j + 1) * 128],
                            qkT[p][base : base + 64, nh * 512 : (nh + 1) * 512],
                            start=True,
                            stop=True,
                        )
                    nc.scalar.activation(
                        eAB[half][:, j, :], psAB[half][:], Exp, scale=SCALE
                    )

        def emit_u(p, eAB):
            # U^T[d, n] = sum_m v_aug[m, d] expT[m, n]; v stationary, expT
            # moving at N=512. Row 64 = softmax denominator (ones column).
            for half in range(2):
                h = 2 * p + half
                e = eAB[half]
                uT_sb = uT_pool.tile([D + 2, T], MMDT, tag="uT", name="uT")
                ups = [
                    accB.tile([96, 512], FP, tag="accB", name="accB")
                    for _ in range(2)
                ]
                for j in range(TC):  # lhsT (v) reused across both halves
                    for nh in range(2):
                        nc.tensor.matmul(
                            ups[nh][:],
                            v65[j][:, h, :],
                            e[:, j, nh * 512 : (nh + 1) * 512],
                            start=(j == 0),
                            stop=(j == TC - 1),
                        )
                for nh in range(2):
                    nc.vector.tensor_copy(
                        uT_sb[:, nh * 512 : (nh + 1) * 512], ups[nh][0 : D + 2, :]
                    )
                # transpose U^T back to token-major, normalize by 1/row64
                for i in range(TC):
                    tps = accA.tile([128, 512], FP, tag="accA", name="accA")
                    tph = tps[:, 0:256].bitcast(MMDT) if fast else tps[:, 0 : D + 2]
                    nc.tensor.transpose(
                        tph[:, 0 : D + 2],
                        uT_sb[:, i * 128 : (i + 1) * 128],
                        identity_h[0 : D + 2, 0 : D + 2],
                    )
                    r = r_pool.tile([128, 1], FP, tag="r", name="r")
                    nc.vector.reciprocal(r[:], tph[:, D : D + 1])
                    nc.vector.tensor_scalar_mul(
                        attn_out[i][:, h * D : (h + 1) * D], tph[:, 0:D], r[:]
                    )

        def emit_aotp(c):
            # head pair c filled attn_out cols c*128:(c+1)*128 == proj lhsT c
            for t in range(TC):
                ps = accA.tile([128, 512], FP, tag="accA", name="accA")
                psh = ps[:, 0:256].bitcast(MMDT) if fast else ps[:, 0:128]
                nc.tensor.transpose(
                    psh[:, 0:128],
                    attn_out[t][:, c * 128 : (c + 1) * 128],
                    identity_h[:],
                )
                nc.any.tensor_copy(aoT[c][:, t * 128 : (t + 1) * 128], psh[:, 0:128])

        def emit_proj():
            # proj: y = attn_outT.T @ w_proj + b
            for t in range(TC):
                y = y_pool.tile([128, C], FP, tag="y", name="y")
                for nh in range(2):
                    ps = accA.tile([128, 512], FP, tag="accA", name="accA")
                    for c in range(CCH):
                        nc.tensor.matmul(
                            ps[:, 0:384],
                            aoT[c][:, t * 128 : (t + 1) * 128],
                            wp[c][:, nh * 384 : (nh + 1) * 384],
                            start=(c == 0),
                            stop=(c == CCH - 1),
                        )
                    nc.vector.tensor_add(
                        y[:, nh * 384 : (nh + 1) * 384],
                        ps[:, 0:384],
                        b_bcast[:, nh * 384 : (nh + 1) * 384],
                    )
                nc.sync.dma_start(outa[t * 128 : (t + 1) * 128, :], y[:])

        # ---- woven emission schedule ----
        f1_order = (0, 6, 1, 7, 2, 8, 3, 9, 4, 10, 5, 11)
        emit_f1(0)
        emit_f1(6)
        eAB_list = []

        def new_pair():
            eAB = [
                exp_pool.tile([128, TC, T], MMDT, tag="expT", name="expT")
                for _ in range(2)
            ]
            eAB_list.append(eAB)
            return eAB

        emit_scores_exp(0, new_pair())
        for t in range(TC):
            emit_f2(t)
        emit_f1(1)
        emit_f1(7)
        emit_scores_exp(1, new_pair())
        for p in range(2, NPAIR):
            emit_f1(f1_order[2 * p])
            emit_f1(f1_order[2 * p + 1])
            emit_scores_exp(p, new_pair())
            emit_u(p - 2, eAB_list[p - 2])
            emit_aotp(p - 2)
        emit_u(NPAIR - 2, eAB_list[NPAIR - 2])
        emit_aotp(NPAIR - 2)
        emit_u(NPAIR - 1, eAB_list[NPAIR - 1])
        emit_aotp(NPAIR - 1)

    nc.finalize()
    return nc


_NC_CACHE = {}


def _get_nc():
    fast = os.environ.get("KERNEL_FAST", "1") == "1"
    if "nc" not in _NC_CACHE:
        _NC_CACHE["nc"] = build(fast=fast)
    return _NC_CACHE["nc"]


def kernel(x, w_qkv, w_proj, b_proj):
    """Full inputs in, full output out. Shards batch across 8 NeuronCores."""
    assert x.shape == (N_CORES, T, C), x.shape
    nc = _get_nc()
    in_maps = [
        {
            "x": np.ascontiguousarray(x[i], dtype=np.float32),
            "w_qkv": np.ascontiguousarray(w_qkv, dtype=np.float32),
            "w_proj": np.ascontiguousarray(w_proj, dtype=np.float32),
            "b_proj": np.ascontiguousarray(b_proj, dtype=np.float32),
        }
        for i in range(N_CORES)
    ]
    res = run_bass_kernel_spmd(nc, in_maps, list(range(N_CORES)))
    return np.stack([res.results[i]["out"] for i in range(N_CORES)], axis=0)


# revision 42
# speedup vs baseline: 1.3741x; 1.0107x over previous
"""Multi-head attention (B=8, N=1024, C=768, H=12) on 8 Trainium2 NeuronCores.

Strategy: pure data parallelism over the batch dimension — each of the 8
cores computes full attention for one batch element; weights are
replicated. No collectives needed.

Per-core dataflow (all matmuls expressed as out = lhsT.T @ rhs on the PE):
  1. xT  = transpose(x)                    (PE identity-transpose, 48 blocks)
  2. qkT = w_qkv[:, :1536].T @ xT          (q,k feature-major, bf16)
     v   = x @ w_qkv[:, 1536:]             (v token-major, bf16, padded slots)
  3. per head pair (2 heads share a 128-row qkT chunk -> row-tiled K=64):
       scoresT[m,n] = k_h @ q_h^T          (lhsT = kT slice, rhs = qT slice)
       expT = exp(scale * scoresT)         (ScalarE; max-subtraction skipped:
                                            |scores*scale| < ~2, exp safe)
       U^T[d,n] += v_aug[m,d] expT[m,n]    (v stationary incl ones column ->
                                            row 64 = softmax denominator)
       transpose U^T per 128-token chunk, multiply by 1/row64 (per-partition)
  4. attn_outT = transpose(attn_out); y = attn_outT.T @ w_proj + b

All matmul operands are bf16 (inputs rounded on load / eviction); all
accumulation is fp32 in PSUM. Single flat pool scope; emission weaves the
qkT production, score pairs, and U phases so ScalarE's exp stream starts
while phase-1 matmuls still run.
"""

import os
import sys

for _p in ("/opt/trn_rl_repo", "/root/.axon_site/_ro/trn_rl_repo"):
    if os.path.isdir(_p) and _p not in sys.path:
        sys.path.append(_p)

from contextlib import ExitStack

import numpy as np

import concourse.bass as bass
import concourse.tile as tile
from concourse import bacc, mybir
from concourse.bass_utils import run_bass_kernel_spmd
from concourse.masks import make_identity

FP = mybir.dt.float32
BF16 = mybir.dt.bfloat16
N_CORES = 8
T = 1024  # tokens per core (batch element)
C = 768
H = 12
D = 64
SCALE = D ** (-0.5)
TC = T // 128  # 8 token chunks
CCH = C // 128  # 6 channel chunks
NPAIR = H // 2  # 6 head pairs

Exp = mybir.ActivationFunctionType.Exp


def build(n_cores: int = N_CORES, fast: bool = True):
    MMDT = BF16 if fast else FP
    nc = bacc.Bacc(
        "TRN2", target_bir_lowering=False, debug=False, num_devices=n_cores
    )
    wdma = nc.gpsimd.dma_start if fast else nc.sync.dma_start
    x = nc.declare_dram_parameter("x", [T, C], FP, isOutput=False)
    w_qkv = nc.declare_dram_parameter("w_qkv", [C, 3 * C], FP, isOutput=False)
    w_proj = nc.declare_dram_parameter("w_proj", [C, C], FP, isOutput=False)
    b_proj = nc.declare_dram_parameter("b_proj", [C], FP, isOutput=False)
    out = nc.declare_dram_parameter("out", [T, C], FP, isOutput=True)

    xa, wqa, wpa, outa = x.ap(), w_qkv.ap(), w_proj.ap(), out.ap()
    ba = b_proj.ap()
    b_bcast_src = bass.AP(tensor=ba.tensor, offset=ba.offset, ap=[[0, 128]] + ba.ap)

    with tile.TileContext(nc) as tc, ExitStack() as ctx:
        # ---- one flat scope: no pool-boundary serialization anywhere ----
        consts = ctx.enter_context(tc.tile_pool(name="consts", bufs=1))
        qk_pool = ctx.enter_context(tc.tile_pool(name="qk", bufs=12))
        v_pool = ctx.enter_context(tc.tile_pool(name="v65", bufs=TC))
        ao_pool = ctx.enter_context(tc.tile_pool(name="attn_out", bufs=TC))
        wp_pool = ctx.enter_context(tc.tile_pool(name="wp", bufs=1))
        y_pool = ctx.enter_context(tc.tile_pool(name="y", bufs=2))
        r_pool = ctx.enter_context(tc.tile_pool(name="r", bufs=4))
        xs_pool = ctx.enter_context(tc.tile_pool(name="xstage", bufs=1))
        xT_pool = ctx.enter_context(tc.tile_pool(name="xT", bufs=CCH))
        wq1_pool = ctx.enter_context(tc.tile_pool(name="wq1", bufs=1))
        wq2_pool = ctx.enter_context(tc.tile_pool(name="wq2", bufs=1))
        exp_pool = ctx.enter_context(tc.tile_pool(name="expT", bufs=4))
        uT_pool = ctx.enter_context(tc.tile_pool(name="uT", bufs=2))
        aoT_pool = ctx.enter_context(tc.tile_pool(name="aoT", bufs=CCH))
        # PSUM: sc 2x2 banks + accA 2x1 + accB 2x1 = 8 banks
        sc_psum = ctx.enter_context(tc.tile_pool(name="sc", bufs=2, space="PSUM"))
        accA = ctx.enter_context(tc.tile_pool(name="accA", bufs=2, space="PSUM"))
        accB = ctx.enter_context(tc.tile_pool(name="accB", bufs=2, space="PSUM"))

        identity = consts.tile([128, 128], FP)
        make_identity(nc, identity)
        identity_h = consts.tile([128, 128], MMDT)
        make_identity(nc, identity_h)

        v65 = [
            v_pool.tile([128, H, 96], MMDT, tag="v65", name="v65")
            for _ in range(TC)
        ]
        attn_out = [
            ao_pool.tile([128, C], MMDT, tag="ao", name="ao") for _ in range(TC)
        ]
        qkT = [qk_pool.tile([128, T], MMDT, tag="qk", name="qk") for _ in range(12)]
        xT = [xT_pool.tile([128, T], MMDT, tag="xT", name="xT") for _ in range(CCH)]
        aoT = [
            aoT_pool.tile([128, T], MMDT, tag="aoT", name="aoT") for _ in range(CCH)
        ]

        # One big casting DMA per input (descriptor fan-out parallelizes
        # across engines; avoids 26 serial SWDGE descriptor generations).
        # Priority: x, then qk weights, then v weights, then proj weights.
        def grouped(src_ap, width, ngrp, col0):
            # view a [ngrp*128, W] dram tensor's width-column slice as
            # [128 partitions, ngrp, width]
            row_step = src_ap.ap[0][0]
            return bass.AP(
                tensor=src_ap.tensor,
                offset=src_ap.offset + col0,
                ap=[[row_step, 128], [128 * row_step, ngrp], [1, width]],
            )

        xs_all = xs_pool.tile([128, TC, C], MMDT, tag="xs", name="xs")
        wdma(xs_all[:], grouped(xa, C, TC, 0))
        wq1_all = wq1_pool.tile([128, CCH, 2 * C], MMDT, tag="wq1", name="wq1")
        wdma(wq1_all[:], grouped(wqa, 2 * C, CCH, 0))
        wq2_all = wq2_pool.tile([128, CCH, C], MMDT, tag="wq2", name="wq2")
        wdma(wq2_all[:], grouped(wqa, C, CCH, 2 * C))
        wp_all = wp_pool.tile([128, CCH, C], MMDT, tag="wp", name="wp")
        wdma(wp_all[:], grouped(wpa, C, CCH, 0))
        b_bcast = consts.tile([128, C], FP)
        nc.sync.dma_start(b_bcast[:], b_bcast_src)
        wq1 = [wq1_all[:, c, :] for c in range(CCH)]
        wq2 = [wq2_all[:, c, :] for c in range(CCH)]
        wp = [wp_all[:, c, :] for c in range(CCH)]

        # x transpose (48 PE identity-transposes)
        for t in range(TC):
            xs = xs_all[:, t, :]
            for c in range(CCH):
                ps = accA.tile([128, 512], FP, tag="accA", name="accA")
                psh = ps[:, 0:256].bitcast(MMDT) if fast else ps[:, 0:128]
                nc.tensor.transpose(
                    psh[:, 0:128], xs[:, c * 128 : (c + 1) * 128], identity_h[:]
                )
                nc.any.tensor_copy(xT[c][:, t * 128 : (t + 1) * 128], psh[:, 0:128])

        def emit_f1(j):
            # qkT[j] = w_qkv[:, j-chunk].T @ x^T
            for nh in range(2):
                ps = accA.tile([128, 512], FP, tag="accA", name="accA")
                for c in range(CCH):
                    nc.tensor.matmul(
                        ps[:],
                        wq1[c][:, j * 128 : (j + 1) * 128],
                        xT[c][:, nh * 512 : (nh + 1) * 512],
                        start=(c == 0),
                        stop=(c == CCH - 1),
                    )
                nc.any.tensor_copy(qkT[j][:, nh * 512 : (nh + 1) * 512], ps[:])

        def emit_f2(t):
            # v[t] = x[t-chunk] @ w_qkv[:, v-cols]; ones column at slot 64
            nc.vector.memset(v65[t][:, :, D:], 1.0)
            for nh in range(2):
                ps = accA.tile([128, 512], FP, tag="accA", name="accA")
                for c in range(CCH):
                    nc.tensor.matmul(
                        ps[:, 0:384],
                        xT[c][:, t * 128 : (t + 1) * 128],
                        wq2[c][:, nh * 384 : (nh + 1) * 384],
                        start=(c == 0),
                        stop=(c == CCH - 1),
                    )
                nc.any.tensor_copy(
                    v65[t][:, nh * 6 : (nh + 1) * 6, 0:D],
                    ps[:, 0:384].rearrange("p (g d) -> p g d", g=6),
                )

        def emit_scores_exp(p, eAB):
            for j in range(TC):  # key-token chunks (m)
                psAB = [
                    sc_psum.tile([128, T], FP, tag="sc", name="sc")
                    for _ in range(2)
                ]
                for half in range(2):
                    base = 64 * half
                    for nh in range(2):
                        nc.tensor.matmul(
                            psAB[half][:, nh * 512 : (nh + 1) * 512],
                            qkT[6 + p][base : base + 64, j * 128 : (j + 1) * 128],
                            qkT[p][base : base + 64, nh * 512 : (nh + 1) * 512],
                            start=True,
                            stop=True,
                        )
                    nc.scalar.activation(
                        eAB[half][:, j, :], psAB[half][:], Exp, scale=SCALE
                    )

        def emit_u(p, eAB):
            # U^T[d, n] = sum_m v_aug[m, d] expT[m, n]; v stationary, expT
            # moving at N=512. Row 64 = softmax denominator (ones column).
            for half in range(2):
                h = 2 * p + half
                e = eAB[half]
                uT_sb = uT_pool.tile([D + 2, T], MMDT, tag="uT", name="uT")
                ups = [
                    accB.tile([96, 512], FP, tag="accB", name="accB")
                    for _ in range(2)
                ]
                for j in range(TC):  # lhsT (v) reused across both halves
                    for nh in range(2):
                        nc.tensor.matmul(
                            ups[nh][:],
                            v65[j][:, h, :],
                            e[:, j, nh * 512 : (nh + 1) * 512],
                            start=(j == 0),
                            stop=(j == TC - 1),
                        )
                for nh in range(2):
                    nc.vector.tensor_copy(
                        uT_sb[:, nh * 512 : (nh + 1) * 512], ups[nh][0 : D + 2, :]
                    )
                # transpose U^T back to token-major, normalize by 1/row64
                for i in range(TC):
                    tps = accA.tile([128, 512], FP, tag="accA", name="accA")
                    tph = tps[:, 0:256].bitcast(MMDT) if fast else tps[:, 0 : D + 2]
                    nc.tensor.transpose(
                        tph[:, 0 : D + 2],
                        uT_sb[:, i * 128 : (i + 1) * 128],
                        identity_h[0 : D + 2, 0 : D + 2],
                    )
                    r = r_pool.tile([128, 1], FP, tag="r", name="r")
                    nc.vector.reciprocal(r[:], tph[:, D : D + 1])
                    nc.vector.tensor_scalar_mul(
                        attn_out[i][:, h * D : (h + 1) * D], tph[:, 0:D], r[:]
                    )

        def emit_aotp(c):
            # head pair c filled attn_out cols c*128:(c+1)*128 == proj lhsT c
            for t in range(TC):
                ps = accA.tile([128, 512], FP, tag="accA", name="accA")
                psh = ps[:, 0:256].bitcast(MMDT) if fast else ps[:, 0:128]
                nc.tensor.transpose(
                    psh[:, 0:128],
                    attn_out[t][:, c * 128 : (c + 1) * 128],
                    identity_h[:],
                )
                nc.any.tensor_copy(aoT[c][:, t * 128 : (t + 1) * 128], psh[:, 0:128])

        def emit_proj():
            # proj: y = attn_outT.T @ w_proj + b
            for t in range(TC):
                y = y_pool.tile([128, C], FP, tag="y", name="y")
                for nh in range(2):
                    ps = accA.tile([128, 512], FP, tag="accA", name="accA")
                    for c in range(CCH):
                        nc.tensor.matmul(
                            ps[:, 0:384],
                            aoT[c][:, t * 128 : (t + 1) * 128],
                            wp[c][:, nh * 384 : (nh + 1) * 384],
                            start=(c == 0),
                            stop=(c == CCH - 1),
                        )
                    nc.vector.tensor_add(
                        y[:, nh * 384 : (nh + 1) * 384],
                        ps[:, 0:384],
                        b_bcast[:, nh * 384 : (nh + 1) * 384],
                    )
                nc.sync.dma_start(outa[t * 128 : (t + 1) * 128, :], y[:])

        # ---- woven emission schedule ----
        f1_order = (0, 6, 1, 7, 2, 8, 3, 9, 4, 10, 5, 11)
        emit_f1(0)
        emit_f1(6)
        eAB_list = []

        def new_pair():
            eAB = [
                exp_pool.tile([128, TC, T], MMDT, tag="expT", name="expT")
                for _ in range(2)
            ]
            eAB_list.append(eAB)
            return eAB

        emit_scores_exp(0, new_pair())
        for t in range(TC):
            emit_f2(t)
        emit_f1(1)
        emit_f1(7)
        emit_scores_exp(1, new_pair())
        for p in range(2, NPAIR):
            emit_f1(f1_order[2 * p])
            emit_f1(f1_order[2 * p + 1])
            emit_scores_exp(p, new_pair())
            emit_u(p - 2, eAB_list[p - 2])
            emit_aotp(p - 2)
        emit_u(NPAIR - 2, eAB_list[NPAIR - 2])
        emit_aotp(NPAIR - 2)
        emit_u(NPAIR - 1, eAB_list[NPAIR - 1])
        emit_aotp(NPAIR - 1)

    nc.finalize()
    return nc


_NC_CACHE = {}


def _get_nc():
    fast = os.environ.get("KERNEL_FAST", "1") == "1"
    if "nc" not in _NC_CACHE:
        _NC_CACHE["nc"] = build(fast=fast)
    return _NC_CACHE["nc"]


def kernel(x, w_qkv, w_proj, b_proj):
    """Full inputs in, full output out. Shards batch across 8 NeuronCores."""
    assert x.shape == (N_CORES, T, C), x.shape
    nc = _get_nc()
    in_maps = [
        {
            "x": np.ascontiguousarray(x[i], dtype=np.float32),
            "w_qkv": np.ascontiguousarray(w_qkv, dtype=np.float32),
            "w_proj": np.ascontiguousarray(w_proj, dtype=np.float32),
            "b_proj": np.ascontiguousarray(b_proj, dtype=np.float32),
        }
        for i in range(N_CORES)
    ]
    res = run_bass_kernel_spmd(nc, in_maps, list(range(N_CORES)))
    return np.stack([res.results[i]["out"] for i in range(N_CORES)], axis=0)
